# revision 1
# baseline (speedup 1.0000x reference)
"""BitAttention (ternary-weight attention with int4/topk-int8 activation quant)
on 8 Trainium2 NeuronCores.

Sharding: tensor-parallel over heads for qkv-proj + SDPA (heads/8 per core),
AllToAll re-shard to token-parallel for the topk+int8 o-projection.

Numerics: quantized values are exact small integers, so qkv/o projections run
as exact integer arithmetic in bf16 matmuls (fp32 PSUM accumulate). Attention
(rope'd q/k real-valued) runs in fp32 matmuls. Softmax exp on ACT. Top-k
per-row threshold found by binary search on the |value| axis; per-token scales
folded into rope tables / exp bias / output scaling.
"""
import math
import numpy as np
import ml_dtypes

# ---------------------------------------------------------------------------
# TileContext patches for this walrus build (single sem-wait per instruction).
# ---------------------------------------------------------------------------
import re as _re
import concourse.mybir as mybir
import concourse.bass as bass
import concourse.tile as tile
from concourse.tile import TileContext, ScopedClock, VectorClock
from concourse.bass_utils import run_bass_kernel_spmd

_carrier_seq = [0]
_orig_add_instruction = TileContext._add_instruction


def _patched_add_instruction(self, inst):
    si = inst.sync_info
    if si is not None and si.on_wait is not None and len(si.on_wait) > 1:
        waits = list(si.on_wait)
        for w in waits[:-1]:
            _carrier_seq[0] += 1
            carrier = mybir.InstEventSemaphore(
                name=f"waitc_{_carrier_seq[0]}_{inst.name}",
                engine=inst.engine,
                ins=[],
                outs=[],
                sync_info=mybir.SyncInfo(on_wait=[w], on_update=[]),
            )
            _orig_add_instruction(self, carrier)
        si.on_wait = [waits[-1]]
        inst.sync_info = si
    _orig_add_instruction(self, inst)


def _clock_ticks(clock):
    m = _re.match(r"VectorClock\((\[.*\])\)", repr(clock))
    return eval(m.group(1))


def _patched_drain_and_barrier(self, tick_clock, wait_clock):
    nc = self.nc
    ticks = _clock_ticks(tick_clock.global_clock)
    n = len(ticks)
    for i, t in enumerate(ticks):
        if t > 0:
            d = nc.sync.drain()
            vci = VectorClock([t if j == i else 0 for j in range(n)])
            wait_clock.add_sem_waits(d.ins, ScopedClock({None: vci}))
    nc.sync.drain()
    nc.all_engine_barrier()
    assert self.sems is not None
    popped = nc._tile_sem_poison_stack.pop()
    assert popped is self._sem_poison
    nc.clear_and_free_semaphores(list(self.sems.allocated().values()))
    nc.all_engine_barrier()


TileContext._add_instruction = _patched_add_instruction
TileContext._drain_and_barrier = _patched_drain_and_barrier

# ---------------------------------------------------------------------------

F32 = mybir.dt.float32
BF16 = mybir.dt.bfloat16
AF = mybir.ActivationFunctionType
ALU = mybir.AluOpType
AX = mybir.AxisListType
MAGIC = 1.5 * 2.0 ** 23
EPS = 1e-5
THETA = 10000.0
TOPK_RATIO = 0.55
NCORES = 8


class Cfg:
    def __init__(self, B=2, T=2048, D=2048, H=16, HD=128, chunk=256, qchunk=256,
                 search_iters=26, no_collectives=False, stop_after=''):
        self.B, self.T, self.D, self.H, self.HD = B, T, D, H, HD
        self.NT = B * T
        self.HPC = H // NCORES            # heads per core
        self.FS = self.HPC * HD           # feature slice per core
        self.chunk = chunk                # phase-A token chunk
        self.qchunk = qchunk              # attention q chunk
        self.TPC = self.NT // NCORES      # tokens per core in phase C
        self.K = max(1, int(TOPK_RATIO * D))
        self.search_iters = search_iters
        self.no_collectives = no_collectives
        self.stop_after = stop_after
        assert self.NT % 128 == 0 and D % 512 == 0 and HD % 2 == 0
        assert T % qchunk == 0 and self.NT % chunk == 0 and chunk % 128 == 0
        assert self.TPC % 128 == 0 and HD <= 128 and self.FS % 128 == 0
        assert D == H * HD


def rope_tables(cfg):
    hd, T = cfg.HD, cfg.T
    inv = 1.0 / THETA ** (np.arange(0, hd, 2, dtype=np.float32) / hd)
    freqs = np.arange(T, dtype=np.float32)[:, None] * inv[None, :]
    emb = np.concatenate([freqs, freqs], axis=1)          # (T, hd)
    cos = np.cos(emb).astype(np.float32)
    sin = np.sin(emb).astype(np.float32)
    cosT = np.concatenate([cos] * cfg.B, 0).T.copy()      # (hd, NT)
    sinT = np.concatenate([sin] * cfg.B, 0).T.copy()
    sin_pm = sinT.copy()
    sin_pm[: hd // 2] = -sin_pm[: hd // 2]                # rotate-half signs
    return np.ascontiguousarray(cosT), np.ascontiguousarray(sin_pm)


def build(cfg: Cfg):
    nc = bass.Bass("TRN2", target_bir_lowering=False, debug=False,
                   num_devices=NCORES)
    NT, D, HD, FS, TPC = cfg.NT, cfg.D, cfg.HD, cfg.FS, cfg.TPC

    x_d = nc.dram_tensor("x", [NT, D], F32, kind="ExternalInput")
    wqT_d = nc.dram_tensor("wqT", [D, FS], F32, kind="ExternalInput")
    wkT_d = nc.dram_tensor("wkT", [D, FS], F32, kind="ExternalInput")
    wvT_d = nc.dram_tensor("wvT", [D, FS], F32, kind="ExternalInput")
    woT_d = nc.dram_tensor("woT", [D, D], F32, kind="ExternalInput")
    cos_d = nc.dram_tensor("cosT", [HD, NT], F32, kind="ExternalInput")
    sin_d = nc.dram_tensor("sinpmT", [HD, NT], F32, kind="ExternalInput")
    idf_d = nc.dram_tensor("idf", [128, 128], F32, kind="ExternalInput")
    idb_d = nc.dram_tensor("idb", [128, 128], BF16, kind="ExternalInput")
    y_d = nc.dram_tensor("y", [TPC, D], F32, kind="ExternalOutput")

    with TileContext(nc, pool_alloc_mode="queue") as tc:
        _body(nc, tc, cfg, x_d, wqT_d, wkT_d, wvT_d, woT_d, cos_d, sin_d,
              idf_d, idb_d, y_d)
    return nc


def _body(nc, tc, cfg, x_d, wqT_d, wkT_d, wvT_d, woT_d, cos_d, sin_d,
          idf_d, idb_d, y_d):
    NT, D, HD, HPC, FS = cfg.NT, cfg.D, cfg.HD, cfg.HPC, cfg.FS
    NTT, NDT = NT // 128, D // 128
    CH = cfg.chunk
    NCH, CTT = NT // CH, CH // 128
    QC, KT, NQC = cfg.qchunk, cfg.T // 128, cfg.T // cfg.qchunk
    TPC, FTQ = cfg.TPC, FS // 128
    SQRT_SCALE = float(1.0 / math.sqrt(HD))
    HH = HD // 2

    with tc.tile_pool(name="persist", bufs=1) as pp, \
         tc.tile_pool(name="dram", bufs=1, space="DRAM") as dramp:
        idf = pp.tile([128, 128], F32)
        nc.sync.dma_start(idf[:], idf_d[:])
        idb = pp.tile([128, 128], BF16)
        nc.sync.dma_start(idb[:], idb_d[:])
        ones_col = pp.tile([128, 1], F32)        # partition sums (lhsT)
        nc.gpsimd.memset(ones_col[:], 1.0)
        ones_row = pp.tile([1, 128], F32)        # partition broadcast (lhsT)
        nc.gpsimd.memset(ones_row[:], 1.0)
        # per token-tile columns (tokens on partitions)
        inv_sx = pp.tile([128, NTT], F32)
        ln_sv = pp.tile([128, NTT], F32)
        rinv_sv = pp.tile([128, NTT], F32)
        ws_s = pp.tile([128, 4], F32)   # bcast weight scales s_w (q,k,v,o)
        ws_r = pp.tile([128, 4], F32)   # bcast 1/(s_w+eps)
        sob = pp.tile([128, 2], F32)    # bcast (s_wo, 1/(s_wo+eps))
        a2a_in = dramp.tile([NT, FS], F32)
        a2a_out = dramp.tile([NT, FS], F32)

        with tc.tile_pool(name="pAB", bufs=1) as pab:
            # roped q/k [FS, NT] fp32; integer v in token layout (fp32 ints)
            qT = [pab.tile([128, NT], F32, tag=f"qT{i}", name=f"qT{i}") for i in range(FTQ)]
            kT = [pab.tile([128, NT], F32, tag=f"kT{i}", name=f"kT{i}") for i in range(FTQ)]
            vtok = [pab.tile([128, FS], F32, tag=f"vtok{i}", name=f"vtok{i}") for i in range(NTT)]

            with tc.tile_pool(name="pW", bufs=1) as pw:
                # ternarized weight slices (bf16 ints), persist through phase A
                wqt = [pw.tile([128, FS], BF16, tag=f"wqt{i}", name=f"wqt{i}") for i in range(NDT)]
                wkt = [pw.tile([128, FS], BF16, tag=f"wkt{i}", name=f"wkt{i}") for i in range(NDT)]
                wvt = [pw.tile([128, FS], BF16, tag=f"wvt{i}", name=f"wvt{i}") for i in range(NDT)]
                _phase_w(nc, tc, cfg, dramp, wqT_d, wkT_d, wvT_d, woT_d,
                         ones_col, ones_row, ws_s, ws_r, wqt, wkt, wvt)
                if cfg.stop_after == 'W':
                    return
                _phase_a(nc, tc, cfg, x_d, cos_d, sin_d, idf, idb, ones_row,
                         ws_s, inv_sx, ln_sv, rinv_sv, wqt, wkt, wvt,
                         qT, kT, vtok)
            if cfg.stop_after == 'A':
                return
            wom = _WoMean(nc, tc, cfg, woT_d, ones_col, ones_row, sob)
            _phase_b(nc, tc, cfg, idf, ones_col, ones_row, ln_sv, rinv_sv,
                     qT, kT, vtok, a2a_in, wom)
            wom.finish()
        if cfg.stop_after == 'B':
            return

        if cfg.no_collectives:
            nc.sync.dma_start(a2a_out[:], a2a_in[:])
        else:
            nc.gpsimd.collective_compute(
                "AllToAll", ALU.bypass, replica_groups=[list(range(NCORES))],
                ins=[a2a_in[:].opt()], outs=[a2a_out[:].opt()])
        _phase_c(nc, tc, cfg, woT_d, idb, ws_s, ws_r, ones_col, ones_row, sob, a2a_out, y_d)


def _phase_w(nc, tc, cfg, dramp, wqT_d, wkT_d, wvT_d, woT_d, ones_col,
             ones_row, ws_s, ws_r, wqt, wkt, wvt):
    D, FS = cfg.D, cfg.FS
    NDT = D // 128
    with tc.tile_pool(name="ph_w", bufs=1) as wp, \
         tc.tile_pool(name="ph_w_ps", bufs=2, space="PSUM") as wps, \
         tc.tile_pool(name="ph_w1", bufs=1) as wp1:
        partials = wp1.tile([1, 4], F32)
        wraw = {}
        for j, wd in enumerate([wqT_d, wkT_d, wvT_d]):
            acc = wp1.tile([128, 1], F32, tag=f"wacc{j}", name=f"wacc{j}")
            nc.gpsimd.memset(acc[:], 0.0)
            for dt in range(NDT):
                t = wp.tile([128, FS], F32, tag=f"wld_{j}_{dt}",
                            name=f"wld_{j}_{dt}")
                nc.sync.dma_start(t[:], wd[dt * 128:(dt + 1) * 128, :])
                wraw[(j, dt)] = t
                r = wp1.tile([128, 1], F32, tag="wred")
                nc.vector.tensor_reduce(r[:], t[:], axis=AX.X, op=ALU.add,
                                        apply_absolute_value=True)
                nc.vector.tensor_tensor(acc[:], acc[:], r[:], op=ALU.add)
            ps = wps.tile([1, 1], F32, tag="w_ps1")
            nc.tensor.matmul(ps[:], acc[:], ones_col[:, 0:1], start=True,
                             stop=True)
            nc.vector.tensor_copy(partials[:, j:j + 1], ps[:])
        nc.gpsimd.memset(partials[:, 3:4], 0.0)
        ar_in = dramp.tile([1, 4], F32)
        ar_out = dramp.tile([1, 4], F32, addr_space="Shared")
        nc.sync.dma_start(ar_in[:], partials[:])
        if cfg.no_collectives:
            nc.sync.dma_start(ar_out[:], ar_in[:])
        else:
            nc.gpsimd.collective_compute(
                "AllReduce", ALU.add, replica_groups=[list(range(NCORES))],
                ins=[ar_in[:].opt()], outs=[ar_out[:].opt()])
        sums = wp1.tile([1, 4], F32)
        nc.sync.dma_start(sums[:], ar_out[:])
        s_row = wp1.tile([1, 4], F32)
        nc.vector.tensor_scalar(s_row[:], sums[:], 1.0 / (float(D) * float(D)),
                                None, op0=ALU.mult)
        r_row = wp1.tile([1, 4], F32)
        nc.vector.tensor_scalar(r_row[:], s_row[:], EPS, None, op0=ALU.add)
        nc.vector.reciprocal(r_row[:], r_row[:])
        ps_b = wps.tile([128, 4], F32, tag="w_psb")
        nc.tensor.matmul(ps_b[:], ones_row[:], s_row[:], start=True, stop=True)
        nc.scalar.copy(ws_s[:], ps_b[:])
        ps_b2 = wps.tile([128, 4], F32, tag="w_psb")
        nc.tensor.matmul(ps_b2[:], ones_row[:], r_row[:], start=True,
                         stop=True)
        nc.scalar.copy(ws_r[:], ps_b2[:])
        # ternarize in place from the resident raw tiles
        for j, dst in enumerate([wqt, wkt, wvt]):
            for dt in range(NDT):
                t = wraw[(j, dt)]
                nc.vector.tensor_scalar(t[:], t[:], ws_r[:, j:j + 1], MAGIC,
                                        op0=ALU.mult, op1=ALU.add)
                nc.vector.tensor_scalar(t[:], t[:], MAGIC, -1.0,
                                        op0=ALU.subtract, op1=ALU.max)
                nc.vector.tensor_scalar(dst[dt][:], t[:], 1.0, None,
                                        op0=ALU.min)


def _phase_a(nc, tc, cfg, x_d, cos_d, sin_d, idf, idb, ones_row, ws_s,
             inv_sx, ln_sv, rinv_sv, wqt, wkt, wvt, qT, kT, vtok):
    D, HD, FS = cfg.D, cfg.HD, cfg.FS
    NDT = D // 128
    CH = cfg.chunk
    NCH, CTT = cfg.NT // CH, CH // 128
    FTQ = FS // 128
    HH = HD // 2
    with tc.tile_pool(name="ph_a", bufs=2) as ap, \
         tc.tile_pool(name="ph_a3", bufs=3) as ap3, \
         tc.tile_pool(name="ph_a_ps", bufs=3, space="PSUM") as aps, \
         tc.tile_pool(name="ph_a_ps3", bufs=4, space="PSUM") as aps3:
        for ch in range(NCH):
            t0 = ch * CH
            xq = [None] * CTT
            for j in range(CTT):
                tt = t0 // 128 + j
                xt = ap.tile([128, D], F32, tag="xload")
                nc.sync.dma_start(xt[:], x_d[tt * 128:(tt + 1) * 128, :])
                m = ap.tile([128, 1], F32, tag="xm")
                nc.vector.tensor_reduce(m[:], xt[:], axis=AX.X, op=ALU.max,
                                        apply_absolute_value=True)
                nc.vector.tensor_scalar(m[:], m[:], EPS, None, op0=ALU.max)
                nc.vector.tensor_scalar(inv_sx[:, tt:tt + 1], m[:], 1.0 / 7.0,
                                        None, op0=ALU.mult)
                sx = ap.tile([128, 1], F32, tag="xs")
                nc.vector.reciprocal(sx[:], m[:])
                nc.vector.tensor_scalar(sx[:], sx[:], 7.0, None, op0=ALU.mult)
                sv = ap.tile([128, 1], F32, tag="xsv")
                nc.vector.tensor_tensor(sv[:], inv_sx[:, tt:tt + 1],
                                        ws_s[:, 2:3], op=ALU.mult)
                nc.scalar.activation(ln_sv[:, tt:tt + 1], sv[:], AF.Ln)
                nc.vector.reciprocal(rinv_sv[:, tt:tt + 1], sv[:])
                xqj = ap.tile([128, D], BF16, tag="xq")
                tmp = ap.tile([128, D], F32, tag="xtmp")
                nc.vector.tensor_scalar(tmp[:], xt[:], sx[:], MAGIC,
                                        op0=ALU.mult, op1=ALU.add)
                nc.vector.tensor_scalar(xqj[:], tmp[:], MAGIC, None,
                                        op0=ALU.subtract)
                xq[j] = xqj
            # transpose xq -> xqT tiles [128 d, CH] (bf16 ints)
            xqT = [None] * NDT
            for dt in range(NDT):
                pst = aps.tile([128, CH], BF16, tag="ps_misc")
                for j in range(CTT):
                    nc.tensor.transpose(pst[:, j * 128:(j + 1) * 128],
                                        xq[j][:, dt * 128:(dt + 1) * 128],
                                        idb[:])
                xqT[dt] = ap.tile([128, CH], BF16, tag=f"xqT{dt}", name=f"xqT{dt}")
                nc.scalar.copy(xqT[dt][:], pst[:])
            # scaled rope tables for this chunk
            cos_c = ap.tile([HD, CH], F32, tag="cos_c")
            nc.sync.dma_start(cos_c[:], cos_d[:, t0:t0 + CH])
            sin_c = ap.tile([HD, CH], F32, tag="sin_c")
            nc.sync.dma_start(sin_c[:], sin_d[:, t0:t0 + CH])
            tabs = {}
            for (wj, nm) in ((0, 'q'), (1, 'k')):
                colp = ap.tile([128, CTT], F32, tag="colp")
                nc.vector.tensor_scalar(colp[:],
                                        inv_sx[:, t0 // 128:t0 // 128 + CTT],
                                        ws_s[:, wj:wj + 1], None, op0=ALU.mult)
                pst = aps.tile([128, 128], F32, tag="ps_misc")
                nc.tensor.transpose(pst[:CTT, :], colp[:], idf[:])
                srow_t = ap.tile([CTT, 128], F32, tag="srowt")
                nc.scalar.copy(srow_t[:], pst[:CTT, :])
                srow = ap.tile([1, CH], F32, tag="srow")
                nc.sync.dma_start(srow[:], srow_t[:])
                psb = aps.tile([HD, CH], F32, tag="ps_misc")
                nc.tensor.matmul(psb[:], ones_row[:, :HD], srow[:],
                                 start=True, stop=True)
                sb = ap.tile([HD, CH], F32, tag=f"sb_{nm}", name=f"sb_{nm}")
                nc.scalar.copy(sb[:], psb[:])
                tc_t = ap.tile([HD, CH], F32, tag=f"tc_{nm}", name=f"tc_{nm}")
                nc.vector.tensor_tensor(tc_t[:], cos_c[:], sb[:], op=ALU.mult)
                ts_t = ap.tile([HD, CH], F32, tag=f"ts_{nm}", name=f"ts_{nm}")
                nc.vector.tensor_tensor(ts_t[:], sin_c[:], sb[:], op=ALU.mult)
                tabs[nm] = (tc_t, ts_t)
            # projections + rope drains
            for wt, nm, dstT in ((wqt, 'q', qT), (wkt, 'k', kT)):
                table_c, table_s = tabs[nm]
                for ft in range(FTQ):
                    ps = aps3.tile([128, CH], F32, tag="ps_proj")
                    for dt in range(NDT):
                        nc.tensor.matmul(ps[:],
                                         wt[dt][:, ft * 128:(ft + 1) * 128],
                                         xqT[dt][:], start=(dt == 0),
                                         stop=(dt == NDT - 1))
                    dtile = dstT[ft]
                    for hb in range(128 // HD):
                        fo = hb * HD
                        t1 = ap3.tile([HD, CH], F32, tag="ropet1")
                        nc.vector.tensor_tensor(t1[:], ps[fo:fo + HD, :],
                                                table_c[:], op=ALU.mult)
                        t2 = ap3.tile([HD, CH], F32, tag="ropet2")
                        nc.vector.tensor_tensor(t2[:HH, :],
                                                ps[fo + HH:fo + HD, :],
                                                table_s[:HH, :], op=ALU.mult)
                        nc.vector.tensor_tensor(t2[HH:, :], ps[fo:fo + HH, :],
                                                table_s[HH:, :], op=ALU.mult)
                        nc.vector.tensor_tensor(dtile[fo:fo + HD, t0:t0 + CH],
                                                t1[:], t2[:], op=ALU.add)
            # v: integer result, drain fp32, transpose to token layout
            for ft in range(FTQ):
                ps = aps3.tile([128, CH], F32, tag="ps_proj")
                for dt in range(NDT):
                    nc.tensor.matmul(ps[:], wvt[dt][:, ft * 128:(ft + 1) * 128],
                                     xqT[dt][:], start=(dt == 0),
                                     stop=(dt == NDT - 1))
                vtmp = ap3.tile([128, CH], F32, tag="vtmp")
                nc.scalar.copy(vtmp[:], ps[:])
                for j in range(CTT):
                    tt = t0 // 128 + j
                    pst = aps.tile([128, 128], F32, tag="ps_misc")
                    nc.tensor.transpose(pst[:], vtmp[:, j * 128:(j + 1) * 128],
                                        idf[:])
                    nc.scalar.copy(vtok[tt][:, ft * 128:(ft + 1) * 128],
                                   pst[:])


class _WoMean:
    """mean|wo| pass, emitted one tile per attention iteration so the DVE/DMA
    work interleaves with phase B instead of serializing before it."""

    def __init__(self, nc, tc, cfg, woT_d, ones_col, ones_row, sob):
        self.nc, self.tc, self.cfg = nc, tc, cfg
        self.woT_d, self.ones_col, self.ones_row, self.sob = (
            woT_d, ones_col, ones_row, sob)
        self.NDT = cfg.D // 128
        self._cms = [tc.tile_pool(name="pwo", bufs=2),
                     tc.tile_pool(name="pwo1", bufs=1),
                     tc.tile_pool(name="pwo_ps", bufs=1, space="PSUM")]
        self.wp = self._cms[0].__enter__()
        self.wp1 = self._cms[1].__enter__()
        self.wps = self._cms[2].__enter__()
        self.acc = self.wp1.tile([128, 1], F32, name="wo_acc")
        nc.gpsimd.memset(self.acc[:], 0.0)
        self.done = 0

    def step(self):
        if self.done >= self.NDT:
            return
        nc, D = self.nc, self.cfg.D
        dt = self.done
        self.done += 1
        t = self.wp.tile([128, D], F32, tag="wo_ld", name="wo_ld")
        nc.sync.dma_start(t[:], self.woT_d[dt * 128:(dt + 1) * 128, :])
        r = self.wp1.tile([128, 1], F32, tag="wo_red", name="wo_red")
        nc.vector.tensor_reduce(r[:], t[:], axis=AX.X, op=ALU.add,
                                apply_absolute_value=True)
        nc.vector.tensor_tensor(self.acc[:], self.acc[:], r[:], op=ALU.add)

    def finish(self):
        while self.done < self.NDT:
            self.step()
        nc, D = self.nc, self.cfg.D
        pss = self.wps.tile([1, 1], F32, tag="wo_ps", name="wo_ps")
        nc.tensor.matmul(pss[:], self.acc[:], self.ones_col[:, 0:1],
                         start=True, stop=True)
        so_s = self.wp1.tile([1, 1], F32, name="so_s")
        nc.vector.tensor_scalar(so_s[:], pss[:], 1.0 / (float(D) * float(D)),
                                None, op0=ALU.mult)
        so_r = self.wp1.tile([1, 1], F32, name="so_r")
        nc.vector.tensor_scalar(so_r[:], so_s[:], EPS, None, op0=ALU.add)
        nc.vector.reciprocal(so_r[:], so_r[:])
        sr2 = self.wp1.tile([1, 2], F32, name="sr2")
        nc.vector.tensor_copy(sr2[:, 0:1], so_s[:])
        nc.vector.tensor_copy(sr2[:, 1:2], so_r[:])
        psb = self.wps.tile([128, 2], F32, tag="wo_ps", name="wo_psb")
        nc.tensor.matmul(psb[:], self.ones_row[:], sr2[:], start=True,
                         stop=True)
        nc.scalar.copy(self.sob[:], psb[:])
        for cm in reversed(self._cms):
            cm.__exit__(None, None, None)


def _phase_b(nc, tc, cfg, idf, ones_col, ones_row, ln_sv, rinv_sv,
             qT, kT, vtok, a2a_in, wom=None):
    HD, HPC = cfg.HD, cfg.HPC
    QC, KT, NQC = cfg.qchunk, cfg.T // 128, cfg.T // cfg.qchunk
    with tc.tile_pool(name="ph_b_p", bufs=2) as bp, \
         tc.tile_pool(name="ph_b_pt", bufs=2) as bpt, \
         tc.tile_pool(name="ph_b_ps_sc", bufs=3, space="PSUM") as ps_sc, \
         tc.tile_pool(name="ph_b_ps_o", bufs=2, space="PSUM") as ps_o, \
         tc.tile_pool(name="ph_b_ps_m", bufs=2, space="PSUM") as ps_m:
        for b in range(cfg.B):
            for hh in range(HPC):
                fo = hh * HD
                ftile, fin = fo // 128, fo % 128
                for qc in range(NQC):
                    q0 = b * cfg.T + qc * QC
                    pT = [bpt.tile([128, QC], F32, tag=f"pT{i}", name=f"pT{i}")
                          for i in range(KT)]
                    den = bp.tile([128, QC], F32, tag="den")
                    outp = ps_o.tile([HD, QC], F32, tag="outp")
                    if wom is not None:
                        wom.step()
                    for kt in range(KT):
                        ktt = (b * cfg.T) // 128 + kt
                        k0 = b * cfg.T + kt * 128
                        ssc = ps_sc.tile([128, QC], F32, tag="ssc")
                        nc.tensor.matmul(
                            ssc[:], kT[ftile][fin:fin + HD, k0:k0 + 128],
                            qT[ftile][fin:fin + HD, q0:q0 + QC],
                            start=True, stop=True)
                        nc.scalar.activation(pT[kt][:], ssc[:], AF.Exp,
                                             bias=ln_sv[:, ktt:ktt + 1],
                                             scale=SQRT_SCALE_OF(cfg))
                        if kt == 0:
                            nc.vector.tensor_scalar(den[:], pT[kt][:],
                                                    rinv_sv[:, ktt:ktt + 1],
                                                    None, op0=ALU.mult)
                        else:
                            nc.vector.scalar_tensor_tensor(
                                den[:], in0=pT[kt][:],
                                scalar=rinv_sv[:, ktt:ktt + 1], in1=den[:],
                                op0=ALU.mult, op1=ALU.add)
                        nc.tensor.matmul(outp[:], vtok[ktt][:, fo:fo + HD],
                                         pT[kt][:], start=(kt == 0),
                                         stop=(kt == KT - 1))
                    dps = ps_m.tile([1, QC], F32, tag="ps_misc")
                    nc.tensor.matmul(dps[:], ones_col[:], den[:], start=True,
                                     stop=True)
                    drow = bp.tile([1, QC], F32, tag="drow")
                    nc.vector.reciprocal(drow[:], dps[:])
                    rdb = ps_m.tile([HD, QC], F32, tag="ps_misc")
                    nc.tensor.matmul(rdb[:], ones_row[:, :HD], drow[:],
                                     start=True, stop=True)
                    osb = bp.tile([HD, QC], F32, tag="osb")
                    nc.scalar.copy(osb[:], outp[:])
                    nc.vector.tensor_tensor(osb[:], osb[:], rdb[:],
                                            op=ALU.mult)
                    for j in range(QC // 128):
                        pst = ps_m.tile([128, HD], F32, tag="ps_misc")
                        nc.tensor.transpose(pst[:],
                                            osb[:, j * 128:(j + 1) * 128],
                                            idf[:])
                        stg = bp.tile([128, HD], F32, tag="stg")
                        nc.scalar.copy(stg[:], pst[:])
                        r0 = q0 + j * 128
                        nc.sync.dma_start(a2a_in[r0:r0 + 128, fo:fo + HD],
                                          stg[:])


def SQRT_SCALE_OF(cfg):
    return float(1.0 / math.sqrt(cfg.HD))


def _phase_c(nc, tc, cfg, woT_d, idb, ws_s, ws_r, ones_col, ones_row, sob, a2a_out, y_d):
    D, TPC = cfg.D, cfg.TPC
    NDT = D // 128
    NTC = TPC // 128
    NFC = D // 512
    with tc.tile_pool(name="pc0", bufs=1) as pc0:
        m8 = pc0.tile([128, NTC], F32)
        lo = pc0.tile([128, NTC], F32)
        s8 = pc0.tile([128, NTC], F32)
        x8 = [pc0.tile([128, D], BF16, tag=f"x8_{j}", name=f"x8_{j}")
              for j in range(NTC)]
        # --- C1: load, abs, threshold search, int8 quant + mask ---
        with tc.tile_pool(name="pc1", bufs=1) as cp1, \
             tc.tile_pool(name="pc1w", bufs=3) as cpw:
            a2a_v = a2a_out[:].rearrange("(s t) f -> t s f", s=NCORES)
            at, absa = [], []
            for j in range(NTC):
                t = cp1.tile([128, D], F32, tag=f"at{j}", name=f"at{j}")
                nc.sync.dma_start(t[:].rearrange("p (s f) -> p s f", s=NCORES),
                                  a2a_v[j * 128:(j + 1) * 128])
                at.append(t)
                ab = cp1.tile([128, D], F32, tag=f"ab{j}", name=f"ab{j}")
                nc.scalar.activation(ab[:], t[:], AF.Abs)
                absa.append(ab)
                nc.vector.tensor_reduce(m8[:, j:j + 1], ab[:], axis=AX.X,
                                        op=ALU.max)
            nc.vector.tensor_scalar(m8[:], m8[:], EPS, None, op0=ALU.max)
            # binary search for the k-th largest |a| per row
            nc.gpsimd.memset(lo[:], 0.0)
            hi = cp1.tile([128, NTC], F32)
            nc.vector.tensor_scalar(hi[:], m8[:], 1.0001, None, op0=ALU.mult)
            mid = cp1.tile([128, NTC], F32)
            nmid = cp1.tile([128, NTC], F32)
            cnt = cp1.tile([128, NTC], F32)
            ge = cp1.tile([128, NTC], F32)
            dif = cp1.tile([128, NTC], F32)
            junk = cp1.tile([128, D], F32)
            junka = cp1.tile([128, D], F32)
            # first iters: upper half of the token tiles counted on ACT via
            # Sign+accum (acc = #above - #below); later iters all on DVE
            # (exact >= semantics near convergence).
            nh = NTC // 2
            act_iters = max(0, cfg.search_iters - 10) if nh else 0
            for it in range(cfg.search_iters):
                nc.vector.tensor_tensor(mid[:], lo[:], hi[:], op=ALU.add)
                nc.vector.tensor_scalar(mid[:], mid[:], 0.5, None, op0=ALU.mult)
                use_act = it < act_iters
                if use_act:
                    nc.vector.tensor_scalar(nmid[:], mid[:], -1.0, None,
                                            op0=ALU.mult)
                for j in range(NTC):
                    if use_act and j >= NTC - nh:
                        nc.scalar.activation(junka[:], absa[j][:], AF.Sign,
                                             bias=nmid[:, j:j + 1],
                                             accum_out=cnt[:, j:j + 1])
                    else:
                        nc.vector.tensor_scalar(junk[:], absa[j][:],
                                                mid[:, j:j + 1], None,
                                                op0=ALU.is_ge, op1=ALU.add,
                                                accum_out=cnt[:, j:j + 1])
                if use_act:
                    nc.vector.tensor_scalar(ge[:, :NTC - nh],
                                            cnt[:, :NTC - nh], float(cfg.K),
                                            None, op0=ALU.is_ge)
                    nc.vector.tensor_scalar(ge[:, NTC - nh:],
                                            cnt[:, NTC - nh:],
                                            float(2 * cfg.K - D), None,
                                            op0=ALU.is_ge)
                else:
                    nc.vector.tensor_scalar(ge[:], cnt[:], float(cfg.K), None,
                                            op0=ALU.is_ge)
                nc.vector.tensor_tensor(dif[:], mid[:], lo[:], op=ALU.subtract)
                nc.vector.tensor_tensor(dif[:], ge[:], dif[:], op=ALU.mult)
                nc.vector.tensor_tensor(lo[:], lo[:], dif[:], op=ALU.add)
                nc.vector.tensor_tensor(dif[:], hi[:], mid[:], op=ALU.subtract)
                nc.vector.tensor_tensor(dif[:], ge[:], dif[:], op=ALU.mult)
                nc.vector.tensor_tensor(hi[:], mid[:], dif[:], op=ALU.add)
            # quantize: x8 = round(a * s8) * (|a| >= lo), s8 = 127/m8
            nc.vector.reciprocal(s8[:], m8[:])
            nc.vector.tensor_scalar(s8[:], s8[:], 127.0, None, op0=ALU.mult)
            for j in range(NTC):
                tmp = cpw.tile([128, D], F32, tag="c_tmp")
                nc.vector.tensor_scalar(tmp[:], at[j][:], s8[:, j:j + 1],
                                        MAGIC, op0=ALU.mult, op1=ALU.add)
                nc.vector.tensor_scalar(tmp[:], tmp[:], MAGIC, None,
                                        op0=ALU.subtract)
                msk = cpw.tile([128, D], F32, tag="c_msk")
                nc.vector.tensor_scalar(msk[:], absa[j][:], lo[:, j:j + 1],
                                        None, op0=ALU.is_ge)
                nc.vector.tensor_tensor(x8[j][:], tmp[:], msk[:], op=ALU.mult)
        # --- C2: transpose x8, ternarize woT, matmul, scale, store ---
        with tc.tile_pool(name="pc2", bufs=1) as cp2, \
             tc.tile_pool(name="pc2w", bufs=3) as cw2, \
             tc.tile_pool(name="pc2_ps", bufs=3, space="PSUM") as cps:
            x8T = []
            for dt in range(NDT):
                pst = cps.tile([128, TPC], BF16, tag="c_pstr")
                for j in range(NTC):
                    nc.tensor.transpose(pst[:, j * 128:(j + 1) * 128],
                                        x8[j][:, dt * 128:(dt + 1) * 128],
                                        idb[:])
                t = cp2.tile([128, TPC], BF16, tag=f"x8T_{dt}",
                             name=f"x8T_{dt}")
                nc.scalar.copy(t[:], pst[:])
                x8T.append(t)
            wot = []
            for dt in range(NDT):
                t2 = cw2.tile([128, D], F32, tag="c_wo_t")
                nc.sync.dma_start(t2[:], woT_d[dt * 128:(dt + 1) * 128, :])
                nc.vector.tensor_scalar(t2[:], t2[:], sob[:, 1:2],
                                        MAGIC, op0=ALU.mult, op1=ALU.add)
                nc.vector.tensor_scalar(t2[:], t2[:], MAGIC, -1.0,
                                        op0=ALU.subtract, op1=ALU.max)
                tb = cp2.tile([128, D], BF16, tag=f"wot_{dt}",
                              name=f"wot_{dt}")
                nc.vector.tensor_scalar(tb[:], t2[:], 1.0, None, op0=ALU.min)
                wot.append(tb)
            # y = (x8 @ wot.T) * s_wo * m8 / 127
            ysc = cp2.tile([128, NTC], F32)
            nc.vector.tensor_scalar(ysc[:], m8[:], sob[:, 0:1], None,
                                    op0=ALU.mult)
            nc.vector.tensor_scalar(ysc[:], ysc[:], 1.0 / 127.0, None,
                                    op0=ALU.mult)
            for j in range(NTC):
                ysb = cw2.tile([128, D], F32, tag="c_y")
                for fc in range(NFC):
                    ps = cps.tile([128, 512], F32, tag="c_psy")
                    for dt in range(NDT):
                        nc.tensor.matmul(ps[:],
                                         x8T[dt][:, j * 128:(j + 1) * 128],
                                         wot[dt][:, fc * 512:(fc + 1) * 512],
                                         start=(dt == 0), stop=(dt == NDT - 1))
                    nc.vector.tensor_scalar(ysb[:, fc * 512:(fc + 1) * 512],
                                            ps[:], ysc[:, j:j + 1], None,
                                            op0=ALU.mult)
                nc.sync.dma_start(y_d[j * 128:(j + 1) * 128, :], ysb[:])


# ---------------------------------------------------------------------------
# Host-side driver
# ---------------------------------------------------------------------------
_CACHED = {}


def _get_nc(cfg):
    key = (cfg.B, cfg.T, cfg.D, cfg.H, cfg.HD, cfg.chunk, cfg.qchunk,
           cfg.search_iters, cfg.no_collectives, cfg.stop_after)
    if key not in _CACHED:
        _CACHED[key] = build(cfg)
    return _CACHED[key]


def run(cfg, x, wq, wk, wv, wo, **kw):
    NT, D, FS = cfg.NT, cfg.D, cfg.FS
    x2 = np.ascontiguousarray(np.asarray(x, np.float32).reshape(NT, D))
    cosT, sinpm = rope_tables(cfg)
    idf = np.eye(128, dtype=np.float32)
    idb = idf.astype(ml_dtypes.bfloat16)
    woT = np.ascontiguousarray(np.asarray(wo, np.float32).T)
    in_maps = []
    for c in range(NCORES):
        fsl = slice(c * FS, (c + 1) * FS)
        in_maps.append({
            "x": x2,
            "wqT": np.ascontiguousarray(np.asarray(wq, np.float32).T[:, fsl]),
            "wkT": np.ascontiguousarray(np.asarray(wk, np.float32).T[:, fsl]),
            "wvT": np.ascontiguousarray(np.asarray(wv, np.float32).T[:, fsl]),
            "woT": woT,
            "cosT": cosT,
            "sinpmT": sinpm,
            "idf": idf,
            "idb": idb,
        })
    nc = _get_nc(cfg)
    res = run_bass_kernel_spmd(nc, in_maps, list(range(NCORES)), **kw)
    y = np.concatenate([res.results[c]["y"] for c in range(NCORES)], 0)
    return y.reshape(cfg.B, cfg.T, cfg.D)


def kernel(x, wq, wk, wv, wo):
    return run(Cfg(), x, wq, wk, wv, wo)


if __name__ == "__main__":
    cfg = Cfg()
    rng = np.random.default_rng(0)
    x = rng.standard_normal((cfg.B, cfg.T, cfg.D)).astype(np.float32)
    ws = [(rng.standard_normal((cfg.D, cfg.D)) * 0.02).astype(np.float32)
          for _ in range(4)]
    y = kernel(x, *ws)
    print("out", y.shape, y.dtype, float(np.abs(y).max()))



# revision 2
# speedup vs baseline: 1.1711x; 1.1711x over previous
"""BitAttention (ternary-weight attention with int4/topk-int8 activation quant)
on 8 Trainium2 NeuronCores — collective-free, token-parallel design.

Measurement in this environment is dominated by per-dispatch overhead, and any
on-device collective costs ~30-45ms of wall time (a 16B AllReduce alone
measures ~97ms vs ~68ms for an empty kernel).  So v2 eliminates ALL
collectives: every core computes 512 output tokens end-to-end, replicating the
k/v projection for its batch (cheap: the whole kernel is ~1ms of compute).

Sharding: core c handles batch b=c//4, token slice r=c%4.  The host rotates
the batch per core so each core's own 512 tokens come first — attention is
bidirectional with no mask, so kv order is irrelevant as long as the rope
tables are rotated identically.  All 8 cores then run one identical program.

Numerics: weights are ternarized on the host (static preprocessing); the
device computes exact-integer projections in bf16 (fp32 PSUM), rope'd q/k in
fp32, attention matmuls in fp32r (full PE rate at N=512), softmax via exp on
ACT with the per-token int4/value scales folded into rope tables and exp bias,
and the topk threshold by 26-iteration binary search (exact >= semantics).
"""
import math
import numpy as np
import ml_dtypes

# ---------------------------------------------------------------------------
# TileContext patches for this walrus build (single sem-wait per instruction).
# ---------------------------------------------------------------------------
import re as _re
import concourse.mybir as mybir
import concourse.bass as bass
import concourse.tile as tile
from concourse.tile import TileContext, ScopedClock, VectorClock
from concourse.bass_utils import run_bass_kernel_spmd

_carrier_seq = [0]
_orig_add_instruction = TileContext._add_instruction


def _patched_add_instruction(self, inst):
    si = inst.sync_info
    if si is not None and si.on_wait is not None and len(si.on_wait) > 1:
        waits = list(si.on_wait)
        for w in waits[:-1]:
            _carrier_seq[0] += 1
            carrier = mybir.InstEventSemaphore(
                name=f"waitc_{_carrier_seq[0]}_{inst.name}",
                engine=inst.engine,
                ins=[],
                outs=[],
                sync_info=mybir.SyncInfo(on_wait=[w], on_update=[]),
            )
            _orig_add_instruction(self, carrier)
        si.on_wait = [waits[-1]]
        inst.sync_info = si
    _orig_add_instruction(self, inst)


def _clock_ticks(clock):
    m = _re.match(r"VectorClock\((\[.*\])\)", repr(clock))
    return eval(m.group(1))


def _patched_drain_and_barrier(self, tick_clock, wait_clock):
    nc = self.nc
    ticks = _clock_ticks(tick_clock.global_clock)
    n = len(ticks)
    for i, t in enumerate(ticks):
        if t > 0:
            d = nc.sync.drain()
            vci = VectorClock([t if j == i else 0 for j in range(n)])
            wait_clock.add_sem_waits(d.ins, ScopedClock({None: vci}))
    nc.sync.drain()
    nc.all_engine_barrier()
    assert self.sems is not None
    popped = nc._tile_sem_poison_stack.pop()
    assert popped is self._sem_poison
    nc.clear_and_free_semaphores(list(self.sems.allocated().values()))
    nc.all_engine_barrier()


TileContext._add_instruction = _patched_add_instruction
TileContext._drain_and_barrier = _patched_drain_and_barrier

# ---------------------------------------------------------------------------

F32 = mybir.dt.float32
F32R = mybir.dt.float32r
BF16 = mybir.dt.bfloat16
AF = mybir.ActivationFunctionType
ALU = mybir.AluOpType
AX = mybir.AxisListType
MAGIC = 1.5 * 2.0 ** 23
EPS = 1e-5
THETA = 10000.0
TOPK_RATIO = 0.55
NCORES = 8


class Cfg:
    def __init__(self, B=2, T=2048, D=2048, H=16, HD=128, search_iters=26,
                 attn_f32r=True, stop_after=''):
        self.B, self.T, self.D, self.H, self.HD = B, T, D, H, HD
        self.NT = B * T
        self.TB = T                        # kv tokens per core (one batch)
        self.TPC = self.NT // NCORES       # own (query/output) tokens per core
        self.NTT = self.TB // 128          # kv token tiles
        self.NDT = D // 128
        self.NTC = self.TPC // 128         # own token tiles
        self.K = max(1, int(TOPK_RATIO * D))
        self.search_iters = search_iters
        self.attn_f32r = attn_f32r
        self.stop_after = stop_after
        assert H * HD == D and HD == 128 and self.TPC % 128 == 0
        assert NCORES == self.B * (T // self.TPC)


def rope_tables(cfg):
    hd, T = cfg.HD, cfg.T
    inv = 1.0 / THETA ** (np.arange(0, hd, 2, dtype=np.float32) / hd)
    freqs = np.arange(T, dtype=np.float32)[:, None] * inv[None, :]
    emb = np.concatenate([freqs, freqs], axis=1)          # (T, hd)
    cos = np.ascontiguousarray(np.cos(emb).astype(np.float32).T)  # (hd, T)
    sin = np.sin(emb).astype(np.float32).T.copy()
    sin[: hd // 2] = -sin[: hd // 2]                      # rotate-half signs
    return cos, np.ascontiguousarray(sin)


def build(cfg: Cfg):
    nc = bass.Bass("TRN2", target_bir_lowering=False, debug=False,
                   num_devices=NCORES)
    TB, TPC, D, HD, H = cfg.TB, cfg.TPC, cfg.D, cfg.HD, cfg.H

    xb_d = nc.dram_tensor("xb", [TB, D], F32, kind="ExternalInput")
    wqS_d = nc.dram_tensor("wqS", [128, H * D // 128 * 128], BF16,
                           kind="ExternalInput")
    wkS_d = nc.dram_tensor("wkS", [128, H * D // 128 * 128], BF16,
                           kind="ExternalInput")
    wvS_d = nc.dram_tensor("wvS", [128, H * D // 128 * 128], BF16,
                           kind="ExternalInput")
    woT_d = nc.dram_tensor("woTt", [D, D], BF16, kind="ExternalInput")
    cosk_d = nc.dram_tensor("cosk", [HD, TB], F32, kind="ExternalInput")
    sink_d = nc.dram_tensor("sink", [HD, TB], F32, kind="ExternalInput")
    cosq_d = nc.dram_tensor("cosq", [HD, TPC], F32, kind="ExternalInput")
    sinq_d = nc.dram_tensor("sinq", [HD, TPC], F32, kind="ExternalInput")
    wsc_d = nc.dram_tensor("wsc", [128, 2], F32, kind="ExternalInput")
    idf_d = nc.dram_tensor("idf", [128, 128], F32, kind="ExternalInput")
    idb_d = nc.dram_tensor("idb", [128, 128], BF16, kind="ExternalInput")
    y_d = nc.dram_tensor("y", [TPC, D], F32, kind="ExternalOutput")

    with TileContext(nc, pool_alloc_mode="queue") as tc, \
         nc.allow_low_precision(reason="f32r attention operands (rounded fp32)"):
        _body(nc, tc, cfg, xb_d, wqS_d, wkS_d, wvS_d, woT_d, cosk_d, sink_d,
              cosq_d, sinq_d, wsc_d, idf_d, idb_d, y_d)
    return nc


def _body(nc, tc, cfg, xb_d, wqS_d, wkS_d, wvS_d, woT_d, cosk_d, sink_d,
          cosq_d, sinq_d, wsc_d, idf_d, idb_d, y_d):
    TB, TPC, D, HD, H = cfg.TB, cfg.TPC, cfg.D, cfg.HD, cfg.H
    NTT, NDT, NTC = cfg.NTT, cfg.NDT, cfg.NTC
    HH = HD // 2
    SQ = float(1.0 / math.sqrt(HD))
    AT_F = F32R if cfg.attn_f32r else F32

    with tc.tile_pool(name="persist", bufs=1) as pp:
        idf = pp.tile([128, 128], F32)
        nc.sync.dma_start(idf[:], idf_d[:])
        idb = pp.tile([128, 128], BF16)
        nc.sync.dma_start(idb[:], idb_d[:])
        wsc = pp.tile([128, 2], F32)
        nc.sync.dma_start(wsc[:], wsc_d[:])
        ones_row = pp.tile([1, 128], F32)
        nc.gpsimd.memset(ones_row[:], 1.0)
        inv_sx = pp.tile([128, NTT], F32)     # (m/7) per kv token
        ln_sv = pp.tile([128, NTT], F32)      # ln(inv_sx * s_wv)
        # 1/(inv_sx * s_wv); written rounded-to-f32r so the den matmul can
        # consume it at full PE rate
        rinv_sv = pp.tile([128, NTT], AT_F)
        # attention output, token layout (own tokens on partitions)
        at = [pp.tile([128, D], F32, tag=f"at{j}", name=f"at{j}")
              for j in range(NTC)]

        with tc.tile_pool(name="xqTp", bufs=1) as xqTp, \
             tc.tile_pool(name="tabp", bufs=1) as tabp:
            xqT = [xqTp.tile([128, TB], BF16, tag=f"xqT{i}", name=f"xqT{i}")
                   for i in range(NDT)]
            tck = tabp.tile([128, TB], F32)
            tsk = tabp.tile([128, TB], F32)
            tcq = tabp.tile([128, TPC], F32)
            tsq = tabp.tile([128, TPC], F32)
            _phase_a(nc, tc, cfg, xb_d, cosk_d, sink_d, cosq_d, sinq_d,
                     idf, idb, wsc, ones_row, inv_sx, ln_sv, rinv_sv,
                     xqT, tck, tsk, tcq, tsq)
            if cfg.stop_after == 'A':
                return
            _phase_b(nc, tc, cfg, wqS_d, wkS_d, wvS_d, idf, ones_row,
                     ln_sv, rinv_sv, xqT, tck, tsk, tcq, tsq, at)
        if cfg.stop_after == 'B':
            return
        _phase_c(nc, tc, cfg, woT_d, idb, wsc, at, y_d)


def _phase_a(nc, tc, cfg, xb_d, cosk_d, sink_d, cosq_d, sinq_d, idf, idb,
             wsc, ones_row, inv_sx, ln_sv, rinv_sv, xqT, tck, tsk, tcq, tsq):
    TB, TPC, D = cfg.TB, cfg.TPC, cfg.D
    NTT, NDT, NTC = cfg.NTT, cfg.NDT, cfg.NTC
    with tc.tile_pool(name="pa", bufs=1) as pa, \
         tc.tile_pool(name="pa1", bufs=2) as pa1, \
         tc.tile_pool(name="pat", bufs=1) as pat, \
         tc.tile_pool(name="pax", bufs=3) as pax, \
         tc.tile_pool(name="pa_ps", bufs=2, space="PSUM") as psa:
        for kc in range(NTT // 4):
            xqs = []
            for j in range(4):
                tt = kc * 4 + j
                xt = pax.tile([128, D], F32, tag="xload", name=f"xt{tt}")
                nc.sync.dma_start(xt[:], xb_d[tt * 128:(tt + 1) * 128, :])
                m = pa1.tile([128, 1], F32, tag="xm")
                nc.vector.tensor_reduce(m[:], xt[:], axis=AX.X, op=ALU.max,
                                        apply_absolute_value=True)
                nc.vector.tensor_scalar(m[:], m[:], EPS, None, op0=ALU.max)
                nc.vector.tensor_scalar(inv_sx[:, tt:tt + 1], m[:], 1.0 / 7.0,
                                        None, op0=ALU.mult)
                sx = pa1.tile([128, 1], F32, tag="xs")
                nc.vector.reciprocal(sx[:], m[:])
                nc.vector.tensor_scalar(sx[:], sx[:], 7.0, None, op0=ALU.mult)
                sv = pa1.tile([128, 1], F32, tag="xsv")
                nc.vector.tensor_scalar(sv[:], inv_sx[:, tt:tt + 1],
                                        wsc[:, 0:1], None, op0=ALU.mult)
                nc.scalar.activation(ln_sv[:, tt:tt + 1], sv[:], AF.Ln)
                nc.vector.reciprocal(rinv_sv[:, tt:tt + 1], sv[:])
                nc.vector.tensor_scalar(xt[:], xt[:], sx[:], MAGIC,
                                        op0=ALU.mult, op1=ALU.add)
                xq = pa.tile([128, D], BF16, tag=f"xq{j}", name=f"xq{tt}")
                nc.vector.tensor_scalar(xq[:], xt[:], MAGIC, None,
                                        op0=ALU.subtract)
                xqs.append(xq)
            for dt in range(NDT):
                pst = psa.tile([128, 512], BF16, tag="pstr")
                for j in range(4):
                    nc.tensor.transpose(pst[:, j * 128:(j + 1) * 128],
                                        xqs[j][:, dt * 128:(dt + 1) * 128],
                                        idb[:])
                nc.scalar.copy(xqT[dt][:, kc * 512:(kc + 1) * 512], pst[:])
        # rope tables scaled by per-token inv_sx (s_wq/s_wk are folded into
        # the host-provided cos/sin tables)
        cosk = pat.tile([128, TB], F32, name="coskt")
        nc.sync.dma_start(cosk[:], cosk_d[:])
        sink = pat.tile([128, TB], F32, name="sinkt")
        nc.sync.dma_start(sink[:], sink_d[:])
        cosq = pat.tile([128, TPC], F32, name="cosqt")
        nc.sync.dma_start(cosq[:], cosq_d[:])
        sinq = pat.tile([128, TPC], F32, name="sinqt")
        nc.sync.dma_start(sinq[:], sinq_d[:])

        pstc = psa.tile([128, 128], F32, tag="ptr2")
        nc.tensor.transpose(pstc[:NTT, :], inv_sx[:], idf[:])
        srow_t = pa1.tile([NTT, 128], F32, tag="srowt", name="srowt")
        nc.scalar.copy(srow_t[:], pstc[:NTT, :])
        srow = pa1.tile([1, TB], F32, tag="srow", name="srow")
        nc.sync.dma_start(srow[:], srow_t[:])
        for ch in range(TB // 512):
            sl = slice(ch * 512, (ch + 1) * 512)
            psb = psa.tile([128, 512], F32, tag="pbc")
            nc.tensor.matmul(psb[:], ones_row[:], srow[:, sl], start=True,
                             stop=True)
            nc.vector.tensor_tensor(tck[:, sl], cosk[:, sl], psb[:],
                                    op=ALU.mult)
            nc.vector.tensor_tensor(tsk[:, sl], sink[:, sl], psb[:],
                                    op=ALU.mult)
            if ch < TPC // 512:
                nc.vector.tensor_tensor(tcq[:, sl], cosq[:, sl], psb[:],
                                        op=ALU.mult)
                nc.vector.tensor_tensor(tsq[:, sl], sinq[:, sl], psb[:],
                                        op=ALU.mult)


def _phase_b(nc, tc, cfg, wqS_d, wkS_d, wvS_d, idf, ones_row, ln_sv, rinv_sv,
             xqT, tck, tsk, tcq, tsq, at):
    TB, TPC, D, HD, H = cfg.TB, cfg.TPC, cfg.D, cfg.HD, cfg.H
    NTT, NDT, NTC = cfg.NTT, cfg.NDT, cfg.NTC
    HH = HD // 2
    SQ = float(1.0 / math.sqrt(HD))
    F = F32R if cfg.attn_f32r else F32

    with tc.tile_pool(name="pw", bufs=2) as pw, \
         tc.tile_pool(name="pb", bufs=2) as pb, \
         tc.tile_pool(name="pbk", bufs=2) as pbk, \
         tc.tile_pool(name="pbv", bufs=1) as pbv, \
         tc.tile_pool(name="ps_p", bufs=1, space="PSUM") as psp, \
         tc.tile_pool(name="ps_s", bufs=2, space="PSUM") as pss, \
         tc.tile_pool(name="ps_o", bufs=1, space="PSUM") as pso, \
         tc.tile_pool(name="ps_m", bufs=1, space="PSUM") as psm:
        for h in range(H):
            fo = h * HD
            wkh = pw.tile([128, TB], BF16, tag="wkh", name=f"wkh{h}")
            nc.sync.dma_start(wkh[:], wkS_d[:, h * TB:(h + 1) * TB])
            wvh = pw.tile([128, TB], BF16, tag="wvh", name=f"wvh{h}")
            nc.sync.dma_start(wvh[:], wvS_d[:, h * TB:(h + 1) * TB])
            wqh = pw.tile([128, TB], BF16, tag="wqh", name=f"wqh{h}")
            nc.sync.dma_start(wqh[:], wqS_d[:, h * TB:(h + 1) * TB])

            kTr = pbk.tile([128, TB], F, tag="kTr", name=f"kTr{h}")
            qTr = pbk.tile([128, TPC], F, tag="qTr", name=f"qTr{h}")
            vtok = [pbv.tile([128, HD], F, tag=f"vt{kt}", name=f"vt{h}_{kt}")
                    for kt in range(NTT)]
            for kc in range(TB // 512):
                sl = slice(kc * 512, (kc + 1) * 512)
                kps = psp.tile([128, 512], F32, tag="kps")
                for dt in range(NDT):
                    nc.tensor.matmul(kps[:], wkh[:, dt * 128:(dt + 1) * 128],
                                     xqT[dt][:, sl], start=(dt == 0),
                                     stop=(dt == NDT - 1))
                t1 = pb.tile([128, 512], F32, tag="ropet1")
                nc.vector.tensor_tensor(t1[:], kps[:], tck[:, sl], op=ALU.mult)
                t2 = pb.tile([128, 512], F32, tag="ropet2")
                nc.vector.tensor_tensor(t2[:HH, :], kps[HH:, :],
                                        tsk[:HH, sl], op=ALU.mult)
                nc.vector.tensor_tensor(t2[HH:, :], kps[:HH, :],
                                        tsk[HH:, sl], op=ALU.mult)
                nc.vector.tensor_tensor(kTr[:, sl], t1[:], t2[:], op=ALU.add)
                vps = psp.tile([128, 512], F32, tag="vps")
                for dt in range(NDT):
                    nc.tensor.matmul(vps[:], wvh[:, dt * 128:(dt + 1) * 128],
                                     xqT[dt][:, sl], start=(dt == 0),
                                     stop=(dt == NDT - 1))
                vsb = pb.tile([128, 512], F32, tag="vsb")
                nc.scalar.copy(vsb[:], vps[:])
                for j in range(4):
                    pst = psm.tile([128, 128], F32, tag="vtr")
                    nc.tensor.transpose(pst[:], vsb[:, j * 128:(j + 1) * 128],
                                        idf[:])
                    nc.scalar.copy(vtok[kc * 4 + j][:], pst[:])
            qps = psp.tile([128, 512], F32, tag="kps")
            for dt in range(NDT):
                nc.tensor.matmul(qps[:], wqh[:, dt * 128:(dt + 1) * 128],
                                 xqT[dt][:, 0:TPC], start=(dt == 0),
                                 stop=(dt == NDT - 1))
            t1q = pb.tile([128, TPC], F32, tag="ropet1")
            nc.vector.tensor_tensor(t1q[:], qps[:], tcq[:], op=ALU.mult)
            t2q = pb.tile([128, TPC], F32, tag="ropet2")
            nc.vector.tensor_tensor(t2q[:HH, :], qps[HH:, :], tsq[:HH, :],
                                    op=ALU.mult)
            nc.vector.tensor_tensor(t2q[HH:, :], qps[:HH, :], tsq[HH:, :],
                                    op=ALU.mult)
            nc.vector.tensor_tensor(qTr[:], t1q[:], t2q[:], op=ALU.add)

            den = pso.tile([1, TPC], F32, tag="den")
            outp = pso.tile([HD, TPC], F32, tag="outp")
            for kt in range(NTT):
                ssc = pss.tile([128, TPC], F32, tag="ssc")
                nc.tensor.matmul(ssc[:], kTr[:, kt * 128:(kt + 1) * 128],
                                 qTr[:], start=True, stop=True)
                pT = pb.tile([128, TPC], F, tag="pT")
                nc.scalar.activation(pT[:], ssc[:], AF.Exp,
                                     bias=ln_sv[:, kt:kt + 1], scale=SQ)
                nc.tensor.matmul(den[:], rinv_sv[:, kt:kt + 1], pT[:],
                                 start=(kt == 0), stop=(kt == NTT - 1))
                nc.tensor.matmul(outp[:], vtok[kt][:], pT[:],
                                 start=(kt == 0), stop=(kt == NTT - 1))
            drow = pb.tile([1, TPC], F32, tag="drow")
            nc.vector.reciprocal(drow[:], den[:])
            rdb = psm.tile([HD, TPC], F32, tag="rdb")
            nc.tensor.matmul(rdb[:], ones_row[:], drow[:], start=True,
                             stop=True)
            osb = pb.tile([HD, TPC], F32, tag="osb")
            nc.scalar.copy(osb[:], outp[:])
            nc.vector.tensor_tensor(osb[:], osb[:], rdb[:], op=ALU.mult)
            for j in range(NTC):
                pst = psm.tile([128, HD], F32, tag="vtr")
                nc.tensor.transpose(pst[:], osb[:, j * 128:(j + 1) * 128],
                                    idf[:])
                nc.scalar.copy(at[j][:, fo:fo + HD], pst[:])


def _phase_c(nc, tc, cfg, woT_d, idb, wsc, at, y_d):
    D, TPC = cfg.D, cfg.TPC
    NDT, NTC = cfg.NDT, cfg.NTC
    NFC = D // 512
    with tc.tile_pool(name="pc0", bufs=1) as pc0, \
         tc.tile_pool(name="pcw", bufs=1) as pcw:
        # start the wo loads right away so they overlap the topk search
        wot = [pcw.tile([128, D], BF16, tag=f"wot{dt}", name=f"wot{dt}")
               for dt in range(NDT)]
        for dt in range(NDT):
            nc.sync.dma_start(wot[dt][:], woT_d[dt * 128:(dt + 1) * 128, :])
        m8 = pc0.tile([128, NTC], F32)
        lo = pc0.tile([128, NTC], F32)
        s8 = pc0.tile([128, NTC], F32)
        x8 = [pc0.tile([128, D], BF16, tag=f"x8_{j}", name=f"x8_{j}")
              for j in range(NTC)]
        # --- C1: abs, threshold search, int8 quant + mask ---
        with tc.tile_pool(name="pc1", bufs=1) as cp1, \
             tc.tile_pool(name="pc1w", bufs=2) as cpw:
            absa = []
            for j in range(NTC):
                ab = cp1.tile([128, D], F32, tag=f"ab{j}", name=f"ab{j}")
                nc.scalar.activation(ab[:], at[j][:], AF.Abs)
                absa.append(ab)
                nc.vector.tensor_reduce(m8[:, j:j + 1], ab[:], axis=AX.X,
                                        op=ALU.max)
            nc.vector.tensor_scalar(m8[:], m8[:], EPS, None, op0=ALU.max)
            nc.gpsimd.memset(lo[:], 0.0)
            hi = cp1.tile([128, NTC], F32)
            nc.vector.tensor_scalar(hi[:], m8[:], 1.0001, None, op0=ALU.mult)
            mid = cp1.tile([128, NTC], F32)
            nmid = cp1.tile([128, NTC], F32)
            cnt = cp1.tile([128, NTC], F32)
            ge = cp1.tile([128, NTC], F32)
            dif = cp1.tile([128, NTC], F32)
            junk = cp1.tile([128, D], F32)
            junka = cp1.tile([128, D], F32)
            # early iters: upper half of the token tiles counted on ACT via
            # Sign+accum (acc = #above - #below); later iters all on DVE
            # (exact >= semantics near convergence).
            nh = NTC // 2
            act_iters = max(0, cfg.search_iters - 10) if nh else 0
            for it in range(cfg.search_iters):
                nc.vector.tensor_tensor(mid[:], lo[:], hi[:], op=ALU.add)
                nc.vector.tensor_scalar(mid[:], mid[:], 0.5, None,
                                        op0=ALU.mult)
                use_act = it < act_iters
                if use_act:
                    nc.vector.tensor_scalar(nmid[:], mid[:], -1.0, None,
                                            op0=ALU.mult)
                for j in range(NTC):
                    if use_act and j >= NTC - nh:
                        nc.scalar.activation(junka[:], absa[j][:], AF.Sign,
                                             bias=nmid[:, j:j + 1],
                                             accum_out=cnt[:, j:j + 1])
                    else:
                        nc.vector.tensor_scalar(junk[:], absa[j][:],
                                                mid[:, j:j + 1], None,
                                                op0=ALU.is_ge, op1=ALU.add,
                                                accum_out=cnt[:, j:j + 1])
                if use_act:
                    nc.vector.tensor_scalar(ge[:, :NTC - nh],
                                            cnt[:, :NTC - nh], float(cfg.K),
                                            None, op0=ALU.is_ge)
                    nc.vector.tensor_scalar(ge[:, NTC - nh:],
                                            cnt[:, NTC - nh:],
                                            float(2 * cfg.K - D), None,
                                            op0=ALU.is_ge)
                else:
                    nc.vector.tensor_scalar(ge[:], cnt[:], float(cfg.K), None,
                                            op0=ALU.is_ge)
                nc.vector.tensor_tensor(dif[:], mid[:], lo[:],
                                        op=ALU.subtract)
                nc.vector.tensor_tensor(dif[:], ge[:], dif[:], op=ALU.mult)
                nc.vector.tensor_tensor(lo[:], lo[:], dif[:], op=ALU.add)
                nc.vector.tensor_tensor(dif[:], hi[:], mid[:],
                                        op=ALU.subtract)
                nc.vector.tensor_tensor(dif[:], ge[:], dif[:], op=ALU.mult)
                nc.vector.tensor_tensor(hi[:], mid[:], dif[:], op=ALU.add)
            # quantize: x8 = round(a * s8) * (|a| >= lo), s8 = 127/m8
            nc.vector.reciprocal(s8[:], m8[:])
            nc.vector.tensor_scalar(s8[:], s8[:], 127.0, None, op0=ALU.mult)
            for j in range(NTC):
                tmp = cpw.tile([128, D], F32, tag="c_tmp")
                nc.vector.tensor_scalar(tmp[:], at[j][:], s8[:, j:j + 1],
                                        MAGIC, op0=ALU.mult, op1=ALU.add)
                nc.vector.tensor_scalar(tmp[:], tmp[:], MAGIC, None,
                                        op0=ALU.subtract)
                msk = cpw.tile([128, D], F32, tag="c_msk")
                nc.vector.tensor_scalar(msk[:], absa[j][:], lo[:, j:j + 1],
                                        None, op0=ALU.is_ge)
                nc.vector.tensor_tensor(x8[j][:], tmp[:], msk[:], op=ALU.mult)
        # --- C2: transpose x8, matmul vs pre-ternarized woT, scale, store ---
        with tc.tile_pool(name="pc2", bufs=1) as cp2, \
             tc.tile_pool(name="pc2w", bufs=3) as cw2, \
             tc.tile_pool(name="pc2_ps", bufs=3, space="PSUM") as cps:
            x8T = []
            for dt in range(NDT):
                pst = cps.tile([128, TPC], BF16, tag="c_pstr")
                for j in range(NTC):
                    nc.tensor.transpose(pst[:, j * 128:(j + 1) * 128],
                                        x8[j][:, dt * 128:(dt + 1) * 128],
                                        idb[:])
                t = cp2.tile([128, TPC], BF16, tag=f"x8T_{dt}",
                             name=f"x8T_{dt}")
                nc.scalar.copy(t[:], pst[:])
                x8T.append(t)
            # y = (x8 @ wot.T) * (s_wo/127) * m8
            ysc = cp2.tile([128, NTC], F32)
            nc.vector.tensor_scalar(ysc[:], m8[:], wsc[:, 1:2], None,
                                    op0=ALU.mult)
            for j in range(NTC):
                ysb = cw2.tile([128, D], F32, tag="c_y")
                for fc in range(NFC):
                    ps = cps.tile([128, 512], F32, tag="c_psy")
                    for dt in range(NDT):
                        nc.tensor.matmul(ps[:],
                                         x8T[dt][:, j * 128:(j + 1) * 128],
                                         wot[dt][:, fc * 512:(fc + 1) * 512],
                                         start=(dt == 0), stop=(dt == NDT - 1))
                    nc.vector.tensor_scalar(ysb[:, fc * 512:(fc + 1) * 512],
                                            ps[:], ysc[:, j:j + 1], None,
                                            op0=ALU.mult)
                nc.sync.dma_start(y_d[j * 128:(j + 1) * 128, :], ysb[:])


# ---------------------------------------------------------------------------
# Host-side driver
# ---------------------------------------------------------------------------
_CACHED = {}


def _get_nc(cfg):
    key = (cfg.B, cfg.T, cfg.D, cfg.H, cfg.HD, cfg.search_iters,
           cfg.attn_f32r, cfg.stop_after)
    if key not in _CACHED:
        _CACHED[key] = build(cfg)
    return _CACHED[key]


def _ternarize(w):
    w = np.asarray(w, np.float32)
    s = np.float32(np.mean(np.abs(w)))
    wi = np.clip(np.round(w / (s + np.float32(EPS))), -1.0, 1.0)
    return s, wi.astype(np.float32)


def _swizzle_qkv(wi, H, HD):
    # w [D_out, D_in] -> wT [D_in, D_out] -> [128, (h t f)] with
    # col ((h*NDT + t)*128 + f) = wT[t*128 + p, h*HD + f]
    D = wi.shape[0]
    wT = np.ascontiguousarray(wi.T)
    NDT = D // 128
    return np.ascontiguousarray(
        wT.reshape(NDT, 128, H, HD).transpose(1, 2, 0, 3).reshape(128, -1)
    ).astype(ml_dtypes.bfloat16)


def prep_inputs(cfg, x, wq, wk, wv, wo):
    B, T, D, H, HD = cfg.B, cfg.T, cfg.D, cfg.H, cfg.HD
    TPC = cfg.TPC
    x = np.asarray(x, np.float32).reshape(B, T, D)
    s_q, wq_i = _ternarize(wq)
    s_k, wk_i = _ternarize(wk)
    s_v, wv_i = _ternarize(wv)
    s_o, wo_i = _ternarize(wo)
    wqS = _swizzle_qkv(wq_i, H, HD)
    wkS = _swizzle_qkv(wk_i, H, HD)
    wvS = _swizzle_qkv(wv_i, H, HD)
    woTt = np.ascontiguousarray(wo_i.T).astype(ml_dtypes.bfloat16)
    cos, sin_pm = rope_tables(cfg)
    idf = np.eye(128, dtype=np.float32)
    idb = idf.astype(ml_dtypes.bfloat16)
    wsc = np.zeros((128, 2), np.float32)
    wsc[:, 0] = s_v
    wsc[:, 1] = s_o / 127.0
    in_maps = []
    for c in range(NCORES):
        b, r = divmod(c, T // TPC)
        perm = (np.arange(T) + r * TPC) % T
        in_maps.append({
            "xb": np.ascontiguousarray(x[b][perm]),
            "wqS": wqS, "wkS": wkS, "wvS": wvS, "woTt": woTt,
            "cosk": np.ascontiguousarray(cos[:, perm] * s_k),
            "sink": np.ascontiguousarray(sin_pm[:, perm] * s_k),
            "cosq": np.ascontiguousarray(cos[:, perm[:TPC]] * s_q),
            "sinq": np.ascontiguousarray(sin_pm[:, perm[:TPC]] * s_q),
            "wsc": wsc, "idf": idf, "idb": idb,
        })
    return in_maps


def run(cfg, x, wq, wk, wv, wo, **kw):
    in_maps = prep_inputs(cfg, x, wq, wk, wv, wo)
    nc = _get_nc(cfg)
    res = run_bass_kernel_spmd(nc, in_maps, list(range(NCORES)), **kw)
    T, TPC, D = cfg.T, cfg.TPC, cfg.D
    y = np.empty((cfg.B, T, D), np.float32)
    for c in range(NCORES):
        b, r = divmod(c, T // TPC)
        y[b, r * TPC:(r + 1) * TPC] = res.results[c]["y"]
    return y


def kernel(x, wq, wk, wv, wo):
    return run(Cfg(), x, wq, wk, wv, wo)


if __name__ == "__main__":
    cfg = Cfg()
    rng = np.random.default_rng(0)
    x = rng.standard_normal((cfg.B, cfg.T, cfg.D)).astype(np.float32)
    ws = [(rng.standard_normal((cfg.D, cfg.D)) * 0.02).astype(np.float32)
          for _ in range(4)]
    y = kernel(x, *ws)
    print("out", y.shape, y.dtype, float(np.abs(y).max()))


# revision 5
# speedup vs baseline: 1.1918x; 1.0177x over previous
"""BitAttention (ternary-weight attention with int4/topk-int8 activation quant)
on 8 Trainium2 NeuronCores — collective-free, token-parallel design.

Measurement in this environment is dominated by per-dispatch overhead, and any
on-device collective costs ~30-45ms of wall time (a 16B AllReduce alone
measures ~97ms vs ~68ms for an empty kernel).  So v2 eliminates ALL
collectives: every core computes 512 output tokens end-to-end, replicating the
k/v projection for its batch (cheap: the whole kernel is ~1ms of compute).

Sharding: core c handles batch b=c//4, token slice r=c%4.  The host rotates
the batch per core so each core's own 512 tokens come first — attention is
bidirectional with no mask, so kv order is irrelevant as long as the rope
tables are rotated identically.  All 8 cores then run one identical program.

Numerics: weights are ternarized on the host (static preprocessing); the
device computes exact-integer projections in bf16 (fp32 PSUM), rope'd q/k in
fp32, attention matmuls in fp32r (full PE rate at N=512), softmax via exp on
ACT with the per-token int4/value scales folded into rope tables and exp bias,
and the topk threshold by 26-iteration binary search (exact >= semantics).
"""
import math
import numpy as np
import ml_dtypes

# ---------------------------------------------------------------------------
# TileContext patches for this walrus build (single sem-wait per instruction).
# ---------------------------------------------------------------------------
import re as _re
import concourse.mybir as mybir
import concourse.bass as bass
import concourse.tile as tile
from concourse.tile import TileContext, ScopedClock, VectorClock
from concourse.bass_utils import run_bass_kernel_spmd

_carrier_seq = [0]
_orig_add_instruction = TileContext._add_instruction


def _patched_add_instruction(self, inst):
    si = inst.sync_info
    if si is not None and si.on_wait is not None and len(si.on_wait) > 1:
        waits = list(si.on_wait)
        for w in waits[:-1]:
            _carrier_seq[0] += 1
            carrier = mybir.InstEventSemaphore(
                name=f"waitc_{_carrier_seq[0]}_{inst.name}",
                engine=inst.engine,
                ins=[],
                outs=[],
                sync_info=mybir.SyncInfo(on_wait=[w], on_update=[]),
            )
            _orig_add_instruction(self, carrier)
        si.on_wait = [waits[-1]]
        inst.sync_info = si
    _orig_add_instruction(self, inst)


def _clock_ticks(clock):
    m = _re.match(r"VectorClock\((\[.*\])\)", repr(clock))
    return eval(m.group(1))


def _patched_drain_and_barrier(self, tick_clock, wait_clock):
    nc = self.nc
    ticks = _clock_ticks(tick_clock.global_clock)
    n = len(ticks)
    for i, t in enumerate(ticks):
        if t > 0:
            d = nc.sync.drain()
            vci = VectorClock([t if j == i else 0 for j in range(n)])
            wait_clock.add_sem_waits(d.ins, ScopedClock({None: vci}))
    nc.sync.drain()
    nc.all_engine_barrier()
    assert self.sems is not None
    popped = nc._tile_sem_poison_stack.pop()
    assert popped is self._sem_poison
    nc.clear_and_free_semaphores(list(self.sems.allocated().values()))
    nc.all_engine_barrier()


TileContext._add_instruction = _patched_add_instruction
TileContext._drain_and_barrier = _patched_drain_and_barrier

# ---------------------------------------------------------------------------

F32 = mybir.dt.float32
F32R = mybir.dt.float32r
BF16 = mybir.dt.bfloat16
AF = mybir.ActivationFunctionType
ALU = mybir.AluOpType
AX = mybir.AxisListType
MAGIC = 1.5 * 2.0 ** 23
EPS = 1e-5
THETA = 10000.0
TOPK_RATIO = 0.55
NCORES = 8


class Cfg:
    def __init__(self, B=2, T=2048, D=2048, H=16, HD=128, search_iters=26,
                 attn_f32r=True, stop_after=''):
        self.B, self.T, self.D, self.H, self.HD = B, T, D, H, HD
        self.NT = B * T
        self.TB = T                        # kv tokens per core (one batch)
        self.TPC = self.NT // NCORES       # own (query/output) tokens per core
        self.NTT = self.TB // 128          # kv token tiles
        self.NDT = D // 128
        self.NTC = self.TPC // 128         # own token tiles
        self.K = max(1, int(TOPK_RATIO * D))
        self.search_iters = search_iters
        self.attn_f32r = attn_f32r
        self.stop_after = stop_after
        assert H * HD == D and HD == 128 and self.TPC % 128 == 0
        assert NCORES == self.B * (T // self.TPC)


def rope_tables(cfg):
    hd, T = cfg.HD, cfg.T
    inv = 1.0 / THETA ** (np.arange(0, hd, 2, dtype=np.float32) / hd)
    freqs = np.arange(T, dtype=np.float32)[:, None] * inv[None, :]
    emb = np.concatenate([freqs, freqs], axis=1)          # (T, hd)
    cos = np.ascontiguousarray(np.cos(emb).astype(np.float32).T)  # (hd, T)
    sin = np.sin(emb).astype(np.float32).T.copy()
    sin[: hd // 2] = -sin[: hd // 2]                      # rotate-half signs
    return cos, np.ascontiguousarray(sin)


def build(cfg: Cfg):
    nc = bass.Bass("TRN2", target_bir_lowering=False, debug=False,
                   num_devices=NCORES)
    TB, TPC, D, HD, H = cfg.TB, cfg.TPC, cfg.D, cfg.HD, cfg.H

    xb_d = nc.dram_tensor("xb", [TB, D], F32, kind="ExternalInput")
    wqS_d = nc.dram_tensor("wqS", [128, H * D // 128 * 128], BF16,
                           kind="ExternalInput")
    wkS_d = nc.dram_tensor("wkS", [128, H * D // 128 * 128], BF16,
                           kind="ExternalInput")
    wvS_d = nc.dram_tensor("wvS", [128, H * D // 128 * 128], BF16,
                           kind="ExternalInput")
    woT_d = nc.dram_tensor("woTt", [D, D], BF16, kind="ExternalInput")
    cosk_d = nc.dram_tensor("cosk", [HD, TB], F32, kind="ExternalInput")
    sink_d = nc.dram_tensor("sink", [HD, TB], F32, kind="ExternalInput")
    cosq_d = nc.dram_tensor("cosq", [HD, TPC], F32, kind="ExternalInput")
    sinq_d = nc.dram_tensor("sinq", [HD, TPC], F32, kind="ExternalInput")
    wsc_d = nc.dram_tensor("wsc", [128, 2], F32, kind="ExternalInput")
    idf_d = nc.dram_tensor("idf", [128, 128], F32, kind="ExternalInput")
    idb_d = nc.dram_tensor("idb", [128, 128], BF16, kind="ExternalInput")
    y_d = nc.dram_tensor("y", [TPC, D], F32, kind="ExternalOutput")

    with TileContext(nc, pool_alloc_mode="queue") as tc, \
         nc.allow_low_precision(reason="f32r attention operands (rounded fp32)"):
        _body(nc, tc, cfg, xb_d, wqS_d, wkS_d, wvS_d, woT_d, cosk_d, sink_d,
              cosq_d, sinq_d, wsc_d, idf_d, idb_d, y_d)
    return nc


def _body(nc, tc, cfg, xb_d, wqS_d, wkS_d, wvS_d, woT_d, cosk_d, sink_d,
          cosq_d, sinq_d, wsc_d, idf_d, idb_d, y_d):
    TB, TPC, D, HD, H = cfg.TB, cfg.TPC, cfg.D, cfg.HD, cfg.H
    NTT, NDT, NTC = cfg.NTT, cfg.NDT, cfg.NTC
    HH = HD // 2
    SQ = float(1.0 / math.sqrt(HD))
    AT_F = F32R if cfg.attn_f32r else F32

    with tc.tile_pool(name="persist", bufs=1) as pp:
        idf = pp.tile([128, 128], F32)
        nc.sync.dma_start(idf[:], idf_d[:])
        idb = pp.tile([128, 128], BF16)
        nc.sync.dma_start(idb[:], idb_d[:])
        wsc = pp.tile([128, 2], F32)
        nc.sync.dma_start(wsc[:], wsc_d[:])
        ones_row = pp.tile([1, 128], F32)
        nc.gpsimd.memset(ones_row[:], 1.0)
        inv_sx = pp.tile([128, NTT], F32)     # (m/7) per kv token
        ln_sv = pp.tile([128, NTT], F32)      # ln(inv_sx * s_wv)
        # 1/(inv_sx * s_wv); written rounded-to-f32r so the den matmul can
        # consume it at full PE rate
        rinv_sv = pp.tile([128, NTT], AT_F)
        # attention output, token layout (own tokens on partitions)
        at = [pp.tile([128, D], F32, tag=f"at{j}", name=f"at{j}")
              for j in range(NTC)]

        with tc.tile_pool(name="xqTp", bufs=1) as xqTp, \
             tc.tile_pool(name="tabp", bufs=1) as tabp:
            xqT = [xqTp.tile([128, TB], BF16, tag=f"xqT{i}", name=f"xqT{i}")
                   for i in range(NDT)]
            tck = tabp.tile([128, TB], F32)
            tsk = tabp.tile([128, TB], F32)
            tcq = tabp.tile([128, TPC], F32)
            tsq = tabp.tile([128, TPC], F32)
            _phase_a(nc, tc, cfg, xb_d, cosk_d, sink_d, cosq_d, sinq_d,
                     idf, idb, wsc, ones_row, inv_sx, ln_sv, rinv_sv,
                     xqT, tck, tsk, tcq, tsq)
            if cfg.stop_after == 'A':
                return
            _phase_b(nc, tc, cfg, wqS_d, wkS_d, wvS_d, idf, ones_row,
                     ln_sv, rinv_sv, xqT, tck, tsk, tcq, tsq, at)
        if cfg.stop_after == 'B':
            return
        _phase_c(nc, tc, cfg, woT_d, idb, wsc, at, y_d)


def _phase_a(nc, tc, cfg, xb_d, cosk_d, sink_d, cosq_d, sinq_d, idf, idb,
             wsc, ones_row, inv_sx, ln_sv, rinv_sv, xqT, tck, tsk, tcq, tsq):
    TB, TPC, D = cfg.TB, cfg.TPC, cfg.D
    NTT, NDT, NTC = cfg.NTT, cfg.NDT, cfg.NTC
    with tc.tile_pool(name="pa", bufs=1) as pa, \
         tc.tile_pool(name="pa1", bufs=2) as pa1, \
         tc.tile_pool(name="pat", bufs=1) as pat, \
         tc.tile_pool(name="pax", bufs=3) as pax, \
         tc.tile_pool(name="pa_ps", bufs=2, space="PSUM") as psa:
        for kc in range(NTT // 4):
            xqs = []
            for j in range(4):
                tt = kc * 4 + j
                xt = pax.tile([128, D], F32, tag="xload", name=f"xt{tt}")
                nc.sync.dma_start(xt[:], xb_d[tt * 128:(tt + 1) * 128, :])
                m = pa1.tile([128, 1], F32, tag="xm")
                nc.vector.tensor_reduce(m[:], xt[:], axis=AX.X, op=ALU.max,
                                        apply_absolute_value=True)
                nc.vector.tensor_scalar(m[:], m[:], EPS, None, op0=ALU.max)
                nc.vector.tensor_scalar(inv_sx[:, tt:tt + 1], m[:], 1.0 / 7.0,
                                        None, op0=ALU.mult)
                sx = pa1.tile([128, 1], F32, tag="xs")
                nc.vector.reciprocal(sx[:], m[:])
                nc.vector.tensor_scalar(sx[:], sx[:], 7.0, None, op0=ALU.mult)
                sv = pa1.tile([128, 1], F32, tag="xsv")
                nc.vector.tensor_scalar(sv[:], inv_sx[:, tt:tt + 1],
                                        wsc[:, 0:1], None, op0=ALU.mult)
                nc.scalar.activation(ln_sv[:, tt:tt + 1], sv[:], AF.Ln)
                nc.vector.reciprocal(rinv_sv[:, tt:tt + 1], sv[:])
                nc.vector.tensor_scalar(xt[:], xt[:], sx[:], MAGIC,
                                        op0=ALU.mult, op1=ALU.add)
                xq = pa.tile([128, D], BF16, tag=f"xq{j}", name=f"xq{tt}")
                nc.vector.tensor_scalar(xq[:], xt[:], MAGIC, None,
                                        op0=ALU.subtract)
                xqs.append(xq)
            for dt in range(NDT):
                pst = psa.tile([128, 512], BF16, tag="pstr")
                for j in range(4):
                    nc.tensor.transpose(pst[:, j * 128:(j + 1) * 128],
                                        xqs[j][:, dt * 128:(dt + 1) * 128],
                                        idb[:])
                nc.scalar.copy(xqT[dt][:, kc * 512:(kc + 1) * 512], pst[:])
        # rope tables scaled by per-token inv_sx (s_wq/s_wk are folded into
        # the host-provided cos/sin tables)
        cosk = pat.tile([128, TB], F32, name="coskt")
        nc.sync.dma_start(cosk[:], cosk_d[:])
        sink = pat.tile([128, TB], F32, name="sinkt")
        nc.sync.dma_start(sink[:], sink_d[:])
        cosq = pat.tile([128, TPC], F32, name="cosqt")
        nc.sync.dma_start(cosq[:], cosq_d[:])
        sinq = pat.tile([128, TPC], F32, name="sinqt")
        nc.sync.dma_start(sinq[:], sinq_d[:])

        pstc = psa.tile([128, 128], F32, tag="ptr2")
        nc.tensor.transpose(pstc[:NTT, :], inv_sx[:], idf[:])
        srow_t = pa1.tile([NTT, 128], F32, tag="srowt", name="srowt")
        nc.scalar.copy(srow_t[:], pstc[:NTT, :])
        srow = pa1.tile([1, TB], F32, tag="srow", name="srow")
        nc.sync.dma_start(srow[:], srow_t[:])
        for ch in range(TB // 512):
            sl = slice(ch * 512, (ch + 1) * 512)
            psb = psa.tile([128, 512], F32, tag="pbc")
            nc.tensor.matmul(psb[:], ones_row[:], srow[:, sl], start=True,
                             stop=True)
            nc.vector.tensor_tensor(tck[:, sl], cosk[:, sl], psb[:],
                                    op=ALU.mult)
            nc.vector.tensor_tensor(tsk[:, sl], sink[:, sl], psb[:],
                                    op=ALU.mult)
            if ch < TPC // 512:
                nc.vector.tensor_tensor(tcq[:, sl], cosq[:, sl], psb[:],
                                        op=ALU.mult)
                nc.vector.tensor_tensor(tsq[:, sl], sinq[:, sl], psb[:],
                                        op=ALU.mult)


def _phase_b(nc, tc, cfg, wqS_d, wkS_d, wvS_d, idf, ones_row, ln_sv, rinv_sv,
             xqT, tck, tsk, tcq, tsq, at):
    TB, TPC, D, HD, H = cfg.TB, cfg.TPC, cfg.D, cfg.HD, cfg.H
    NTT, NDT, NTC = cfg.NTT, cfg.NDT, cfg.NTC
    HH = HD // 2
    SQ = float(1.0 / math.sqrt(HD))
    F = F32R if cfg.attn_f32r else F32

    with tc.tile_pool(name="pw", bufs=2) as pw, \
         tc.tile_pool(name="pb", bufs=2) as pb, \
         tc.tile_pool(name="pbk", bufs=2) as pbk, \
         tc.tile_pool(name="pbv", bufs=1) as pbv, \
         tc.tile_pool(name="ps_p", bufs=1, space="PSUM") as psp, \
         tc.tile_pool(name="ps_s", bufs=2, space="PSUM") as pss, \
         tc.tile_pool(name="ps_o", bufs=1, space="PSUM") as pso, \
         tc.tile_pool(name="ps_m", bufs=1, space="PSUM") as psm:
        for h in range(H):
            fo = h * HD
            wkh = pw.tile([128, TB], BF16, tag="wkh", name=f"wkh{h}")
            nc.sync.dma_start(wkh[:], wkS_d[:, h * TB:(h + 1) * TB])
            wvh = pw.tile([128, TB], BF16, tag="wvh", name=f"wvh{h}")
            nc.sync.dma_start(wvh[:], wvS_d[:, h * TB:(h + 1) * TB])
            wqh = pw.tile([128, TB], BF16, tag="wqh", name=f"wqh{h}")
            nc.sync.dma_start(wqh[:], wqS_d[:, h * TB:(h + 1) * TB])

            kTr = pbk.tile([128, TB], F, tag="kTr", name=f"kTr{h}")
            qTr = pbk.tile([128, TPC], F, tag="qTr", name=f"qTr{h}")
            # v in token layout, 4 token-tiles packed per tile so the PSUM
            # transpose drain is one big ACT copy instead of four small ones
            vt4 = [pbv.tile([128, 512], F, tag=f"vt{kc}", name=f"vt{h}_{kc}")
                   for kc in range(TB // 512)]
            for kc in range(TB // 512):
                sl = slice(kc * 512, (kc + 1) * 512)
                kps = psp.tile([128, 512], F32, tag="kps")
                for dt in range(NDT):
                    nc.tensor.matmul(kps[:], wkh[:, dt * 128:(dt + 1) * 128],
                                     xqT[dt][:, sl], start=(dt == 0),
                                     stop=(dt == NDT - 1))
                t1 = pb.tile([128, 512], F32, tag="ropet1")
                nc.vector.tensor_tensor(t1[:], kps[:], tck[:, sl], op=ALU.mult)
                t2 = pb.tile([128, 512], F32, tag="ropet2")
                nc.vector.tensor_tensor(t2[:HH, :], kps[HH:, :],
                                        tsk[:HH, sl], op=ALU.mult)
                nc.vector.tensor_tensor(t2[HH:, :], kps[:HH, :],
                                        tsk[HH:, sl], op=ALU.mult)
                nc.vector.tensor_tensor(kTr[:, sl], t1[:], t2[:], op=ALU.add)
                vps = psp.tile([128, 512], F32, tag="vps")
                for dt in range(NDT):
                    nc.tensor.matmul(vps[:], wvh[:, dt * 128:(dt + 1) * 128],
                                     xqT[dt][:, sl], start=(dt == 0),
                                     stop=(dt == NDT - 1))
                vsb = pb.tile([128, 512], F32, tag="vsb")
                nc.scalar.copy(vsb[:], vps[:])
                pstv = psm.tile([128, 512], F32, tag="vtr")
                for j in range(4):
                    nc.tensor.transpose(pstv[:, j * 128:(j + 1) * 128],
                                        vsb[:, j * 128:(j + 1) * 128],
                                        idf[:])
                nc.scalar.copy(vt4[kc][:], pstv[:])
            qps = psp.tile([128, 512], F32, tag="kps")
            for dt in range(NDT):
                nc.tensor.matmul(qps[:], wqh[:, dt * 128:(dt + 1) * 128],
                                 xqT[dt][:, 0:TPC], start=(dt == 0),
                                 stop=(dt == NDT - 1))
            t1q = pb.tile([128, TPC], F32, tag="ropet1")
            nc.vector.tensor_tensor(t1q[:], qps[:], tcq[:], op=ALU.mult)
            t2q = pb.tile([128, TPC], F32, tag="ropet2")
            nc.vector.tensor_tensor(t2q[:HH, :], qps[HH:, :], tsq[:HH, :],
                                    op=ALU.mult)
            nc.vector.tensor_tensor(t2q[HH:, :], qps[:HH, :], tsq[HH:, :],
                                    op=ALU.mult)
            nc.vector.tensor_tensor(qTr[:], t1q[:], t2q[:], op=ALU.add)

            den = pso.tile([1, TPC], F32, tag="den")
            outp = pso.tile([HD, TPC], F32, tag="outp")
            for kt in range(NTT):
                ssc = pss.tile([128, TPC], F32, tag="ssc")
                nc.tensor.matmul(ssc[:], kTr[:, kt * 128:(kt + 1) * 128],
                                 qTr[:], start=True, stop=True)
                pT = pb.tile([128, TPC], F, tag="pT")
                nc.scalar.activation(pT[:], ssc[:], AF.Exp,
                                     bias=ln_sv[:, kt:kt + 1], scale=SQ)
                nc.tensor.matmul(den[:], rinv_sv[:, kt:kt + 1], pT[:],
                                 start=(kt == 0), stop=(kt == NTT - 1))
                nc.tensor.matmul(
                    outp[:], vt4[kt // 4][:, (kt % 4) * 128:(kt % 4 + 1) * 128],
                    pT[:], start=(kt == 0), stop=(kt == NTT - 1))
            drow = pb.tile([1, TPC], F32, tag="drow")
            nc.vector.reciprocal(drow[:], den[:])
            rdb = psm.tile([HD, TPC], F32, tag="rdb")
            nc.tensor.matmul(rdb[:], ones_row[:], drow[:], start=True,
                             stop=True)
            osb = pb.tile([HD, TPC], F32, tag="osb")
            nc.scalar.copy(osb[:], outp[:])
            nc.vector.tensor_tensor(osb[:], osb[:], rdb[:], op=ALU.mult)
            for j in range(NTC):
                pst = psm.tile([128, HD], F32, tag="vtr")
                nc.tensor.transpose(pst[:], osb[:, j * 128:(j + 1) * 128],
                                    idf[:])
                nc.scalar.copy(at[j][:, fo:fo + HD], pst[:])


def _phase_c(nc, tc, cfg, woT_d, idb, wsc, at, y_d):
    D, TPC = cfg.D, cfg.TPC
    NDT, NTC = cfg.NDT, cfg.NTC
    NFC = D // 512
    with tc.tile_pool(name="pc0", bufs=1) as pc0, \
         tc.tile_pool(name="pcw", bufs=1) as pcw:
        # start the wo loads right away so they overlap the topk search
        wot = [pcw.tile([128, D], BF16, tag=f"wot{dt}", name=f"wot{dt}")
               for dt in range(NDT)]
        for dt in range(NDT):
            nc.sync.dma_start(wot[dt][:], woT_d[dt * 128:(dt + 1) * 128, :])
        m8 = pc0.tile([128, NTC], F32)
        lo = pc0.tile([128, NTC], F32)
        s8 = pc0.tile([128, NTC], F32)
        x8 = [pc0.tile([128, D], BF16, tag=f"x8_{j}", name=f"x8_{j}")
              for j in range(NTC)]
        # --- C1: abs, threshold search, int8 quant + mask ---
        with tc.tile_pool(name="pc1", bufs=1) as cp1, \
             tc.tile_pool(name="pc1w", bufs=2) as cpw:
            absa = []
            for j in range(NTC):
                ab = cp1.tile([128, D], F32, tag=f"ab{j}", name=f"ab{j}")
                nc.scalar.activation(ab[:], at[j][:], AF.Abs)
                absa.append(ab)
                nc.vector.tensor_reduce(m8[:, j:j + 1], ab[:], axis=AX.X,
                                        op=ALU.max)
            nc.vector.tensor_scalar(m8[:], m8[:], EPS, None, op0=ALU.max)
            nc.gpsimd.memset(lo[:], 0.0)
            hi = cp1.tile([128, NTC], F32)
            nc.vector.tensor_scalar(hi[:], m8[:], 1.0001, None, op0=ALU.mult)
            mid = cp1.tile([128, NTC], F32)
            nmid = cp1.tile([128, NTC], F32)
            cnt = cp1.tile([128, NTC], F32)
            ge = cp1.tile([128, NTC], F32)
            dif = cp1.tile([128, NTC], F32)
            junk = cp1.tile([128, D], F32)
            junka = cp1.tile([128, D], F32)
            # early iters: upper half of the token tiles counted on ACT via
            # Sign+accum (acc = #above - #below); later iters all on DVE
            # (exact >= semantics near convergence).
            nh = NTC // 2
            act_iters = max(0, cfg.search_iters - 10) if nh else 0
            for it in range(cfg.search_iters):
                nc.vector.tensor_tensor(mid[:], lo[:], hi[:], op=ALU.add)
                nc.vector.tensor_scalar(mid[:], mid[:], 0.5, None,
                                        op0=ALU.mult)
                use_act = it < act_iters
                if use_act:
                    nc.vector.tensor_scalar(nmid[:], mid[:], -1.0, None,
                                            op0=ALU.mult)
                for j in range(NTC):
                    if use_act and j >= NTC - nh:
                        nc.scalar.activation(junka[:], absa[j][:], AF.Sign,
                                             bias=nmid[:, j:j + 1],
                                             accum_out=cnt[:, j:j + 1])
                    else:
                        nc.vector.tensor_scalar(junk[:], absa[j][:],
                                                mid[:, j:j + 1], None,
                                                op0=ALU.is_ge, op1=ALU.add,
                                                accum_out=cnt[:, j:j + 1])
                if use_act:
                    nc.vector.tensor_scalar(ge[:, :NTC - nh],
                                            cnt[:, :NTC - nh], float(cfg.K),
                                            None, op0=ALU.is_ge)
                    nc.vector.tensor_scalar(ge[:, NTC - nh:],
                                            cnt[:, NTC - nh:],
                                            float(2 * cfg.K - D), None,
                                            op0=ALU.is_ge)
                else:
                    nc.vector.tensor_scalar(ge[:], cnt[:], float(cfg.K), None,
                                            op0=ALU.is_ge)
                nc.vector.tensor_tensor(dif[:], mid[:], lo[:],
                                        op=ALU.subtract)
                nc.vector.tensor_tensor(dif[:], ge[:], dif[:], op=ALU.mult)
                nc.vector.tensor_tensor(lo[:], lo[:], dif[:], op=ALU.add)
                nc.vector.tensor_tensor(dif[:], hi[:], mid[:],
                                        op=ALU.subtract)
                nc.vector.tensor_tensor(dif[:], ge[:], dif[:], op=ALU.mult)
                nc.vector.tensor_tensor(hi[:], mid[:], dif[:], op=ALU.add)
            # quantize: x8 = round(a * s8) * (|a| >= lo), s8 = 127/m8
            nc.vector.reciprocal(s8[:], m8[:])
            nc.vector.tensor_scalar(s8[:], s8[:], 127.0, None, op0=ALU.mult)
            for j in range(NTC):
                tmp = cpw.tile([128, D], F32, tag="c_tmp")
                nc.vector.tensor_scalar(tmp[:], at[j][:], s8[:, j:j + 1],
                                        MAGIC, op0=ALU.mult, op1=ALU.add)
                nc.vector.tensor_scalar(tmp[:], tmp[:], MAGIC, None,
                                        op0=ALU.subtract)
                msk = cpw.tile([128, D], F32, tag="c_msk")
                nc.vector.tensor_scalar(msk[:], absa[j][:], lo[:, j:j + 1],
                                        None, op0=ALU.is_ge)
                nc.vector.tensor_tensor(x8[j][:], tmp[:], msk[:], op=ALU.mult)
        # --- C2: transpose x8, matmul vs pre-ternarized woT, scale, store ---
        with tc.tile_pool(name="pc2", bufs=1) as cp2, \
             tc.tile_pool(name="pc2w", bufs=3) as cw2, \
             tc.tile_pool(name="pc2_ps", bufs=3, space="PSUM") as cps:
            x8T = []
            for dt in range(NDT):
                pst = cps.tile([128, TPC], BF16, tag="c_pstr")
                for j in range(NTC):
                    nc.tensor.transpose(pst[:, j * 128:(j + 1) * 128],
                                        x8[j][:, dt * 128:(dt + 1) * 128],
                                        idb[:])
                t = cp2.tile([128, TPC], BF16, tag=f"x8T_{dt}",
                             name=f"x8T_{dt}")
                nc.scalar.copy(t[:], pst[:])
                x8T.append(t)
            # y = (x8 @ wot.T) * (s_wo/127) * m8
            ysc = cp2.tile([128, NTC], F32)
            nc.vector.tensor_scalar(ysc[:], m8[:], wsc[:, 1:2], None,
                                    op0=ALU.mult)
            for j in range(NTC):
                ysb = cw2.tile([128, D], F32, tag="c_y")
                for fc in range(NFC):
                    ps = cps.tile([128, 512], F32, tag="c_psy")
                    for dt in range(NDT):
                        nc.tensor.matmul(ps[:],
                                         x8T[dt][:, j * 128:(j + 1) * 128],
                                         wot[dt][:, fc * 512:(fc + 1) * 512],
                                         start=(dt == 0), stop=(dt == NDT - 1))
                    nc.vector.tensor_scalar(ysb[:, fc * 512:(fc + 1) * 512],
                                            ps[:], ysc[:, j:j + 1], None,
                                            op0=ALU.mult)
                nc.sync.dma_start(y_d[j * 128:(j + 1) * 128, :], ysb[:])


# ---------------------------------------------------------------------------
# Host-side driver
# ---------------------------------------------------------------------------
_CACHED = {}


def _get_nc(cfg):
    key = (cfg.B, cfg.T, cfg.D, cfg.H, cfg.HD, cfg.search_iters,
           cfg.attn_f32r, cfg.stop_after)
    if key not in _CACHED:
        _CACHED[key] = build(cfg)
    return _CACHED[key]


def _ternarize(w):
    w = np.asarray(w, np.float32)
    s = np.float32(np.mean(np.abs(w)))
    wi = np.clip(np.round(w / (s + np.float32(EPS))), -1.0, 1.0)
    return s, wi.astype(np.float32)


def _swizzle_qkv(wi, H, HD):
    # w [D_out, D_in] -> wT [D_in, D_out] -> [128, (h t f)] with
    # col ((h*NDT + t)*128 + f) = wT[t*128 + p, h*HD + f]
    D = wi.shape[0]
    wT = np.ascontiguousarray(wi.T)
    NDT = D // 128
    return np.ascontiguousarray(
        wT.reshape(NDT, 128, H, HD).transpose(1, 2, 0, 3).reshape(128, -1)
    ).astype(ml_dtypes.bfloat16)


def prep_inputs(cfg, x, wq, wk, wv, wo):
    B, T, D, H, HD = cfg.B, cfg.T, cfg.D, cfg.H, cfg.HD
    TPC = cfg.TPC
    x = np.asarray(x, np.float32).reshape(B, T, D)
    s_q, wq_i = _ternarize(wq)
    s_k, wk_i = _ternarize(wk)
    s_v, wv_i = _ternarize(wv)
    s_o, wo_i = _ternarize(wo)
    wqS = _swizzle_qkv(wq_i, H, HD)
    wkS = _swizzle_qkv(wk_i, H, HD)
    wvS = _swizzle_qkv(wv_i, H, HD)
    woTt = np.ascontiguousarray(wo_i.T).astype(ml_dtypes.bfloat16)
    cos, sin_pm = rope_tables(cfg)
    idf = np.eye(128, dtype=np.float32)
    idb = idf.astype(ml_dtypes.bfloat16)
    wsc = np.zeros((128, 2), np.float32)
    wsc[:, 0] = s_v
    wsc[:, 1] = s_o / 127.0
    in_maps = []
    for c in range(NCORES):
        b, r = divmod(c, T // TPC)
        perm = (np.arange(T) + r * TPC) % T
        in_maps.append({
            "xb": np.ascontiguousarray(x[b][perm]),
            "wqS": wqS, "wkS": wkS, "wvS": wvS, "woTt": woTt,
            "cosk": np.ascontiguousarray(cos[:, perm] * s_k),
            "sink": np.ascontiguousarray(sin_pm[:, perm] * s_k),
            "cosq": np.ascontiguousarray(cos[:, perm[:TPC]] * s_q),
            "sinq": np.ascontiguousarray(sin_pm[:, perm[:TPC]] * s_q),
            "wsc": wsc, "idf": idf, "idb": idb,
        })
    return in_maps


def run(cfg, x, wq, wk, wv, wo, **kw):
    in_maps = prep_inputs(cfg, x, wq, wk, wv, wo)
    nc = _get_nc(cfg)
    res = run_bass_kernel_spmd(nc, in_maps, list(range(NCORES)), **kw)
    T, TPC, D = cfg.T, cfg.TPC, cfg.D
    y = np.empty((cfg.B, T, D), np.float32)
    for c in range(NCORES):
        b, r = divmod(c, T // TPC)
        y[b, r * TPC:(r + 1) * TPC] = res.results[c]["y"]
    return y


def kernel(x, wq, wk, wv, wo):
    return run(Cfg(), x, wq, wk, wv, wo)


if __name__ == "__main__":
    cfg = Cfg()
    rng = np.random.default_rng(0)
    x = rng.standard_normal((cfg.B, cfg.T, cfg.D)).astype(np.float32)
    ws = [(rng.standard_normal((cfg.D, cfg.D)) * 0.02).astype(np.float32)
          for _ in range(4)]
    y = kernel(x, *ws)
    print("out", y.shape, y.dtype, float(np.abs(y).max()))


# revision 6
# speedup vs baseline: 2.3211x; 1.9475x over previous
"""BitAttention (ternary-weight attention with int4/topk-int8 activation quant)
on 8 Trainium2 NeuronCores — collective-free, token-parallel design.

Measurement in this environment is dominated by per-dispatch overhead, and any
on-device collective costs ~30-45ms of wall time (a 16B AllReduce alone
measures ~97ms vs ~68ms for an empty kernel).  So v2 eliminates ALL
collectives: every core computes 512 output tokens end-to-end, replicating the
k/v projection for its batch (cheap: the whole kernel is ~1ms of compute).

Sharding: core c handles batch b=c//4, token slice r=c%4.  The host rotates
the batch per core so each core's own 512 tokens come first — attention is
bidirectional with no mask, so kv order is irrelevant as long as the rope
tables are rotated identically.  All 8 cores then run one identical program.

Numerics: weights are ternarized on the host (static preprocessing); the
device computes exact-integer projections in bf16 (fp32 PSUM), rope'd q/k in
fp32, attention matmuls in fp32r (full PE rate at N=512), softmax via exp on
ACT with the per-token int4/value scales folded into rope tables and exp bias,
and the topk threshold by 26-iteration binary search (exact >= semantics).
"""
import math
import numpy as np
import ml_dtypes

# ---------------------------------------------------------------------------
# TileContext patches for this walrus build (single sem-wait per instruction).
# ---------------------------------------------------------------------------
import re as _re
import concourse.mybir as mybir
import concourse.bass as bass
import concourse.tile as tile
from concourse.tile import TileContext, ScopedClock, VectorClock
from concourse.bass_utils import run_bass_kernel_spmd

_carrier_seq = [0]
_orig_add_instruction = TileContext._add_instruction


def _patched_add_instruction(self, inst):
    si = inst.sync_info
    if si is not None and si.on_wait is not None and len(si.on_wait) > 1:
        waits = list(si.on_wait)
        for w in waits[:-1]:
            _carrier_seq[0] += 1
            carrier = mybir.InstEventSemaphore(
                name=f"waitc_{_carrier_seq[0]}_{inst.name}",
                engine=inst.engine,
                ins=[],
                outs=[],
                sync_info=mybir.SyncInfo(on_wait=[w], on_update=[]),
            )
            _orig_add_instruction(self, carrier)
        si.on_wait = [waits[-1]]
        inst.sync_info = si
    _orig_add_instruction(self, inst)


def _clock_ticks(clock):
    m = _re.match(r"VectorClock\((\[.*\])\)", repr(clock))
    return eval(m.group(1))


def _patched_drain_and_barrier(self, tick_clock, wait_clock):
    nc = self.nc
    ticks = _clock_ticks(tick_clock.global_clock)
    n = len(ticks)
    for i, t in enumerate(ticks):
        if t > 0:
            d = nc.sync.drain()
            vci = VectorClock([t if j == i else 0 for j in range(n)])
            wait_clock.add_sem_waits(d.ins, ScopedClock({None: vci}))
    nc.sync.drain()
    nc.all_engine_barrier()
    assert self.sems is not None
    popped = nc._tile_sem_poison_stack.pop()
    assert popped is self._sem_poison
    nc.clear_and_free_semaphores(list(self.sems.allocated().values()))
    nc.all_engine_barrier()


TileContext._add_instruction = _patched_add_instruction
TileContext._drain_and_barrier = _patched_drain_and_barrier

# ---------------------------------------------------------------------------

F32 = mybir.dt.float32
F32R = mybir.dt.float32r
BF16 = mybir.dt.bfloat16
F8 = mybir.dt.float8e4
DR = mybir.MatmulPerfMode.DoubleRow
AF = mybir.ActivationFunctionType
ALU = mybir.AluOpType
AX = mybir.AxisListType
MAGIC = 1.5 * 2.0 ** 23
EPS = 1e-5
THETA = 10000.0
TOPK_RATIO = 0.55
NCORES = 8


class Cfg:
    def __init__(self, B=2, T=2048, D=2048, H=16, HD=128, search_iters=26,
                 attn_f32r=True, stop_after=''):
        self.B, self.T, self.D, self.H, self.HD = B, T, D, H, HD
        self.NT = B * T
        self.TB = T                        # kv tokens per core (one batch)
        self.TPC = self.NT // NCORES       # own (query/output) tokens per core
        self.NTT = self.TB // 128          # kv token tiles
        self.NDT = D // 128
        self.NTC = self.TPC // 128         # own token tiles
        self.K = max(1, int(TOPK_RATIO * D))
        self.search_iters = search_iters
        self.attn_f32r = attn_f32r
        self.stop_after = stop_after
        assert H * HD == D and HD == 128 and self.TPC % 128 == 0
        assert NCORES == self.B * (T // self.TPC)


def rope_tables(cfg):
    hd, T = cfg.HD, cfg.T
    inv = 1.0 / THETA ** (np.arange(0, hd, 2, dtype=np.float32) / hd)
    freqs = np.arange(T, dtype=np.float32)[:, None] * inv[None, :]
    emb = np.concatenate([freqs, freqs], axis=1)          # (T, hd)
    cos = np.ascontiguousarray(np.cos(emb).astype(np.float32).T)  # (hd, T)
    sin = np.sin(emb).astype(np.float32).T.copy()
    sin[: hd // 2] = -sin[: hd // 2]                      # rotate-half signs
    return cos, np.ascontiguousarray(sin)


def build(cfg: Cfg):
    nc = bass.Bass("TRN2", target_bir_lowering=False, debug=False,
                   num_devices=NCORES)
    TB, TPC, D, HD, H = cfg.TB, cfg.TPC, cfg.D, cfg.HD, cfg.H

    xb_d = nc.dram_tensor("xb", [TB, D], F32, kind="ExternalInput")
    wqS_d = nc.dram_tensor("wqS", [128, H * D // 128 * 128], F8,
                           kind="ExternalInput")
    wkS_d = nc.dram_tensor("wkS", [128, H * D // 128 * 128], F8,
                           kind="ExternalInput")
    wvS_d = nc.dram_tensor("wvS", [128, H * D // 128 * 128], F8,
                           kind="ExternalInput")
    woT_d = nc.dram_tensor("woTt", [D, D], BF16, kind="ExternalInput")
    cosk_d = nc.dram_tensor("cosk", [HD, TB], F32, kind="ExternalInput")
    sink_d = nc.dram_tensor("sink", [HD, TB], F32, kind="ExternalInput")
    cosq_d = nc.dram_tensor("cosq", [HD, TPC], F32, kind="ExternalInput")
    sinq_d = nc.dram_tensor("sinq", [HD, TPC], F32, kind="ExternalInput")
    wsc_d = nc.dram_tensor("wsc", [128, 2], F32, kind="ExternalInput")
    idf_d = nc.dram_tensor("idf", [128, 128], F32, kind="ExternalInput")
    idb_d = nc.dram_tensor("idb", [128, 128], BF16, kind="ExternalInput")
    y_d = nc.dram_tensor("y", [TPC, D], F32, kind="ExternalOutput")

    with TileContext(nc, pool_alloc_mode="queue") as tc, \
         nc.allow_low_precision(reason="f32r attention operands (rounded fp32)"):
        _body(nc, tc, cfg, xb_d, wqS_d, wkS_d, wvS_d, woT_d, cosk_d, sink_d,
              cosq_d, sinq_d, wsc_d, idf_d, idb_d, y_d)
    return nc


def _body(nc, tc, cfg, xb_d, wqS_d, wkS_d, wvS_d, woT_d, cosk_d, sink_d,
          cosq_d, sinq_d, wsc_d, idf_d, idb_d, y_d):
    TB, TPC, D, HD, H = cfg.TB, cfg.TPC, cfg.D, cfg.HD, cfg.H
    NTT, NDT, NTC = cfg.NTT, cfg.NDT, cfg.NTC
    HH = HD // 2
    SQ = float(1.0 / math.sqrt(HD))
    AT_F = F32R if cfg.attn_f32r else F32

    with tc.tile_pool(name="persist", bufs=1) as pp:
        idf = pp.tile([128, 128], F32)
        nc.sync.dma_start(idf[:], idf_d[:])
        idb = pp.tile([128, 128], BF16)
        nc.sync.dma_start(idb[:], idb_d[:])
        wsc = pp.tile([128, 2], F32)
        nc.sync.dma_start(wsc[:], wsc_d[:])
        ones_row = pp.tile([1, 128], F32)
        nc.gpsimd.memset(ones_row[:], 1.0)
        inv_sx = pp.tile([128, NTT], F32)     # (m/7) per kv token
        ln_sv = pp.tile([128, NTT], F32)      # ln(inv_sx * s_wv)
        # 1/(inv_sx * s_wv); written rounded-to-f32r so the den matmul can
        # consume it at full PE rate
        rinv_sv = pp.tile([128, NTT], AT_F)
        # attention output, token layout (own tokens on partitions)
        at = [pp.tile([128, D], F32, tag=f"at{j}", name=f"at{j}")
              for j in range(NTC)]

        with tc.tile_pool(name="xqTp", bufs=1) as xqTp, \
             tc.tile_pool(name="tabp", bufs=1) as tabp:
            # fp8 pair tiles: dim1 = dt parity within pair (DoubleRow K-pair)
            xqT = [xqTp.tile([128, 2, TB], F8, tag=f"xqT{i}", name=f"xqT{i}")
                   for i in range(NDT // 2)]
            tck = tabp.tile([128, TB], F32)
            tsk = tabp.tile([128, TB], F32)
            tcq = tabp.tile([128, TPC], F32)
            tsq = tabp.tile([128, TPC], F32)
            _phase_a(nc, tc, cfg, xb_d, cosk_d, sink_d, cosq_d, sinq_d,
                     idf, idb, wsc, ones_row, inv_sx, ln_sv, rinv_sv,
                     xqT, tck, tsk, tcq, tsq)
            if cfg.stop_after == 'A':
                return
            _phase_b(nc, tc, cfg, wqS_d, wkS_d, wvS_d, idf, ones_row,
                     ln_sv, rinv_sv, xqT, tck, tsk, tcq, tsq, at)
        if cfg.stop_after == 'B':
            return
        _phase_c(nc, tc, cfg, woT_d, idb, wsc, at, y_d)


def _phase_a(nc, tc, cfg, xb_d, cosk_d, sink_d, cosq_d, sinq_d, idf, idb,
             wsc, ones_row, inv_sx, ln_sv, rinv_sv, xqT, tck, tsk, tcq, tsq):
    TB, TPC, D = cfg.TB, cfg.TPC, cfg.D
    NTT, NDT, NTC = cfg.NTT, cfg.NDT, cfg.NTC
    with tc.tile_pool(name="pa", bufs=1) as pa, \
         tc.tile_pool(name="pa1", bufs=2) as pa1, \
         tc.tile_pool(name="pat", bufs=1) as pat, \
         tc.tile_pool(name="pax", bufs=3) as pax, \
         tc.tile_pool(name="pa_ps", bufs=2, space="PSUM") as psa:
        for kc in range(NTT // 4):
            xqs = []
            for j in range(4):
                tt = kc * 4 + j
                xt = pax.tile([128, D], F32, tag="xload", name=f"xt{tt}")
                nc.sync.dma_start(xt[:], xb_d[tt * 128:(tt + 1) * 128, :])
                m = pa1.tile([128, 1], F32, tag="xm")
                nc.vector.tensor_reduce(m[:], xt[:], axis=AX.X, op=ALU.max,
                                        apply_absolute_value=True)
                nc.vector.tensor_scalar(m[:], m[:], EPS, None, op0=ALU.max)
                nc.vector.tensor_scalar(inv_sx[:, tt:tt + 1], m[:], 1.0 / 7.0,
                                        None, op0=ALU.mult)
                sx = pa1.tile([128, 1], F32, tag="xs")
                nc.vector.reciprocal(sx[:], m[:])
                nc.vector.tensor_scalar(sx[:], sx[:], 7.0, None, op0=ALU.mult)
                sv = pa1.tile([128, 1], F32, tag="xsv")
                nc.vector.tensor_scalar(sv[:], inv_sx[:, tt:tt + 1],
                                        wsc[:, 0:1], None, op0=ALU.mult)
                nc.scalar.activation(ln_sv[:, tt:tt + 1], sv[:], AF.Ln)
                nc.vector.reciprocal(rinv_sv[:, tt:tt + 1], sv[:])
                nc.vector.tensor_scalar(xt[:], xt[:], sx[:], MAGIC,
                                        op0=ALU.mult, op1=ALU.add)
                xq = pa.tile([128, D], BF16, tag=f"xq{j}", name=f"xq{tt}")
                nc.vector.tensor_scalar(xq[:], xt[:], MAGIC, None,
                                        op0=ALU.subtract)
                xqs.append(xq)
            for dt in range(NDT):
                pst = psa.tile([128, 512], BF16, tag="pstr")
                for j in range(4):
                    nc.tensor.transpose(pst[:, j * 128:(j + 1) * 128],
                                        xqs[j][:, dt * 128:(dt + 1) * 128],
                                        idb[:])
                nc.scalar.copy(
                    xqT[dt // 2][:, dt % 2, kc * 512:(kc + 1) * 512], pst[:])
        # rope tables scaled by per-token inv_sx (s_wq/s_wk are folded into
        # the host-provided cos/sin tables)
        cosk = pat.tile([128, TB], F32, name="coskt")
        nc.sync.dma_start(cosk[:], cosk_d[:])
        sink = pat.tile([128, TB], F32, name="sinkt")
        nc.sync.dma_start(sink[:], sink_d[:])
        cosq = pat.tile([128, TPC], F32, name="cosqt")
        nc.sync.dma_start(cosq[:], cosq_d[:])
        sinq = pat.tile([128, TPC], F32, name="sinqt")
        nc.sync.dma_start(sinq[:], sinq_d[:])

        pstc = psa.tile([128, 128], F32, tag="ptr2")
        nc.tensor.transpose(pstc[:NTT, :], inv_sx[:], idf[:])
        srow_t = pa1.tile([NTT, 128], F32, tag="srowt", name="srowt")
        nc.scalar.copy(srow_t[:], pstc[:NTT, :])
        srow = pa1.tile([1, TB], F32, tag="srow", name="srow")
        nc.sync.dma_start(srow[:], srow_t[:])
        for ch in range(TB // 512):
            sl = slice(ch * 512, (ch + 1) * 512)
            psb = psa.tile([128, 512], F32, tag="pbc")
            nc.tensor.matmul(psb[:], ones_row[:], srow[:, sl], start=True,
                             stop=True)
            nc.vector.tensor_tensor(tck[:, sl], cosk[:, sl], psb[:],
                                    op=ALU.mult)
            nc.vector.tensor_tensor(tsk[:, sl], sink[:, sl], psb[:],
                                    op=ALU.mult)
            if ch < TPC // 512:
                nc.vector.tensor_tensor(tcq[:, sl], cosq[:, sl], psb[:],
                                        op=ALU.mult)
                nc.vector.tensor_tensor(tsq[:, sl], sinq[:, sl], psb[:],
                                        op=ALU.mult)


def _phase_b(nc, tc, cfg, wqS_d, wkS_d, wvS_d, idf, ones_row, ln_sv, rinv_sv,
             xqT, tck, tsk, tcq, tsq, at):
    TB, TPC, D, HD, H = cfg.TB, cfg.TPC, cfg.D, cfg.HD, cfg.H
    NTT, NDT, NTC = cfg.NTT, cfg.NDT, cfg.NTC
    HH = HD // 2
    SQ = float(1.0 / math.sqrt(HD))
    F = F32R if cfg.attn_f32r else F32

    with tc.tile_pool(name="pw", bufs=2) as pw, \
         tc.tile_pool(name="pb", bufs=2) as pb, \
         tc.tile_pool(name="pbk", bufs=2) as pbk, \
         tc.tile_pool(name="pbv", bufs=1) as pbv, \
         tc.tile_pool(name="ps_p", bufs=1, space="PSUM") as psp, \
         tc.tile_pool(name="ps_s", bufs=2, space="PSUM") as pss, \
         tc.tile_pool(name="ps_o", bufs=1, space="PSUM") as pso, \
         tc.tile_pool(name="ps_m", bufs=1, space="PSUM") as psm:
        for h in range(H):
            fo = h * HD
            wkh = pw.tile([128, TB], F8, tag="wkh", name=f"wkh{h}")
            nc.sync.dma_start(wkh[:], wkS_d[:, h * TB:(h + 1) * TB])
            wvh = pw.tile([128, TB], F8, tag="wvh", name=f"wvh{h}")
            nc.sync.dma_start(wvh[:], wvS_d[:, h * TB:(h + 1) * TB])
            wqh = pw.tile([128, TB], F8, tag="wqh", name=f"wqh{h}")
            nc.sync.dma_start(wqh[:], wqS_d[:, h * TB:(h + 1) * TB])

            def w3(wt, p):
                # [128, 2, 128]: DoubleRow K-pair (dt = 2p+ko) x feature
                return wt[:, p * 256:(p + 1) * 256].rearrange(
                    "a (ko f) -> a ko f", ko=2)

            kTr = pbk.tile([128, TB], F, tag="kTr", name=f"kTr{h}")
            qTr = pbk.tile([128, TPC], F, tag="qTr", name=f"qTr{h}")
            # v in token layout, 4 token-tiles packed per tile so the PSUM
            # transpose drain is one big ACT copy instead of four small ones
            vt4 = [pbv.tile([128, 512], F, tag=f"vt{kc}", name=f"vt{h}_{kc}")
                   for kc in range(TB // 512)]
            for kc in range(TB // 512):
                sl = slice(kc * 512, (kc + 1) * 512)
                kps = psp.tile([128, 512], F32, tag="kps")
                for p in range(NDT // 2):
                    nc.tensor.matmul(kps[:], w3(wkh, p), xqT[p][:, :, sl],
                                     start=(p == 0), stop=(p == NDT // 2 - 1),
                                     perf_mode=DR)
                t1 = pb.tile([128, 512], F32, tag="ropet1")
                nc.vector.tensor_tensor(t1[:], kps[:], tck[:, sl], op=ALU.mult)
                t2 = pb.tile([128, 512], F32, tag="ropet2")
                nc.vector.tensor_tensor(t2[:HH, :], kps[HH:, :],
                                        tsk[:HH, sl], op=ALU.mult)
                nc.vector.tensor_tensor(t2[HH:, :], kps[:HH, :],
                                        tsk[HH:, sl], op=ALU.mult)
                nc.vector.tensor_tensor(kTr[:, sl], t1[:], t2[:], op=ALU.add)
                vps = psp.tile([128, 512], F32, tag="vps")
                for p in range(NDT // 2):
                    nc.tensor.matmul(vps[:], w3(wvh, p), xqT[p][:, :, sl],
                                     start=(p == 0), stop=(p == NDT // 2 - 1),
                                     perf_mode=DR)
                vsb = pb.tile([128, 512], F32, tag="vsb")
                nc.scalar.copy(vsb[:], vps[:])
                pstv = psm.tile([128, 512], F32, tag="vtr")
                for j in range(4):
                    nc.tensor.transpose(pstv[:, j * 128:(j + 1) * 128],
                                        vsb[:, j * 128:(j + 1) * 128],
                                        idf[:])
                nc.scalar.copy(vt4[kc][:], pstv[:])
            qps = psp.tile([128, 512], F32, tag="kps")
            for p in range(NDT // 2):
                nc.tensor.matmul(qps[:], w3(wqh, p), xqT[p][:, :, 0:TPC],
                                 start=(p == 0), stop=(p == NDT // 2 - 1),
                                 perf_mode=DR)
            t1q = pb.tile([128, TPC], F32, tag="ropet1")
            nc.vector.tensor_tensor(t1q[:], qps[:], tcq[:], op=ALU.mult)
            t2q = pb.tile([128, TPC], F32, tag="ropet2")
            nc.vector.tensor_tensor(t2q[:HH, :], qps[HH:, :], tsq[:HH, :],
                                    op=ALU.mult)
            nc.vector.tensor_tensor(t2q[HH:, :], qps[:HH, :], tsq[HH:, :],
                                    op=ALU.mult)
            nc.vector.tensor_tensor(qTr[:], t1q[:], t2q[:], op=ALU.add)

            den = pso.tile([1, TPC], F32, tag="den")
            outp = pso.tile([HD, TPC], F32, tag="outp")
            for kt in range(NTT):
                ssc = pss.tile([128, TPC], F32, tag="ssc")
                nc.tensor.matmul(ssc[:], kTr[:, kt * 128:(kt + 1) * 128],
                                 qTr[:], start=True, stop=True)
                pT = pb.tile([128, TPC], F, tag="pT")
                nc.scalar.activation(pT[:], ssc[:], AF.Exp,
                                     bias=ln_sv[:, kt:kt + 1], scale=SQ)
                nc.tensor.matmul(den[:], rinv_sv[:, kt:kt + 1], pT[:],
                                 start=(kt == 0), stop=(kt == NTT - 1))
                nc.tensor.matmul(
                    outp[:], vt4[kt // 4][:, (kt % 4) * 128:(kt % 4 + 1) * 128],
                    pT[:], start=(kt == 0), stop=(kt == NTT - 1))
            drow = pb.tile([1, TPC], F32, tag="drow")
            nc.vector.reciprocal(drow[:], den[:])
            rdb = psm.tile([HD, TPC], F32, tag="rdb")
            nc.tensor.matmul(rdb[:], ones_row[:], drow[:], start=True,
                             stop=True)
            osb = pb.tile([HD, TPC], F32, tag="osb")
            nc.scalar.copy(osb[:], outp[:])
            nc.vector.tensor_tensor(osb[:], osb[:], rdb[:], op=ALU.mult)
            for j in range(NTC):
                pst = psm.tile([128, HD], F32, tag="vtr")
                nc.tensor.transpose(pst[:], osb[:, j * 128:(j + 1) * 128],
                                    idf[:])
                nc.scalar.copy(at[j][:, fo:fo + HD], pst[:])


def _phase_c(nc, tc, cfg, woT_d, idb, wsc, at, y_d):
    D, TPC = cfg.D, cfg.TPC
    NDT, NTC = cfg.NDT, cfg.NTC
    NFC = D // 512
    with tc.tile_pool(name="pc0", bufs=1) as pc0, \
         tc.tile_pool(name="pcw", bufs=1) as pcw:
        # start the wo loads right away so they overlap the topk search
        wot = [pcw.tile([128, D], BF16, tag=f"wot{dt}", name=f"wot{dt}")
               for dt in range(NDT)]
        for dt in range(NDT):
            nc.sync.dma_start(wot[dt][:], woT_d[dt * 128:(dt + 1) * 128, :])
        m8 = pc0.tile([128, NTC], F32)
        lo = pc0.tile([128, NTC], F32)
        s8 = pc0.tile([128, NTC], F32)
        x8 = [pc0.tile([128, D], BF16, tag=f"x8_{j}", name=f"x8_{j}")
              for j in range(NTC)]
        # --- C1: abs, threshold search, int8 quant + mask ---
        with tc.tile_pool(name="pc1", bufs=1) as cp1, \
             tc.tile_pool(name="pc1w", bufs=1) as cpw:
            absa = []
            for j in range(NTC):
                ab = cp1.tile([128, D], F32, tag=f"ab{j}", name=f"ab{j}")
                nc.scalar.activation(ab[:], at[j][:], AF.Abs)
                absa.append(ab)
                nc.vector.tensor_reduce(m8[:, j:j + 1], ab[:], axis=AX.X,
                                        op=ALU.max)
            nc.vector.tensor_scalar(m8[:], m8[:], EPS, None, op0=ALU.max)
            nc.gpsimd.memset(lo[:], 0.0)
            hi = cp1.tile([128, NTC], F32)
            nc.vector.tensor_scalar(hi[:], m8[:], 1.0001, None, op0=ALU.mult)
            mid = cp1.tile([128, NTC], F32)
            nmid = cp1.tile([128, NTC], F32)
            cnt = cp1.tile([128, NTC], F32)
            ge = cp1.tile([128, NTC], F32)
            dif = cp1.tile([128, NTC], F32)
            junk = cp1.tile([128, D], F32)
            junka = cp1.tile([128, D], F32)
            # early iters: upper half of the token tiles counted on ACT via
            # Sign+accum (acc = #above - #below); later iters all on DVE
            # (exact >= semantics near convergence).
            nh = NTC // 2
            act_iters = max(0, cfg.search_iters - 10) if nh else 0
            for it in range(cfg.search_iters):
                nc.vector.tensor_tensor(mid[:], lo[:], hi[:], op=ALU.add)
                nc.vector.tensor_scalar(mid[:], mid[:], 0.5, None,
                                        op0=ALU.mult)
                use_act = it < act_iters
                if use_act:
                    nc.vector.tensor_scalar(nmid[:], mid[:], -1.0, None,
                                            op0=ALU.mult)
                for j in range(NTC):
                    if use_act and j >= NTC - nh:
                        nc.scalar.activation(junka[:], absa[j][:], AF.Sign,
                                             bias=nmid[:, j:j + 1],
                                             accum_out=cnt[:, j:j + 1])
                    else:
                        nc.vector.tensor_scalar(junk[:], absa[j][:],
                                                mid[:, j:j + 1], None,
                                                op0=ALU.is_ge, op1=ALU.add,
                                                accum_out=cnt[:, j:j + 1])
                if use_act:
                    nc.vector.tensor_scalar(ge[:, :NTC - nh],
                                            cnt[:, :NTC - nh], float(cfg.K),
                                            None, op0=ALU.is_ge)
                    nc.vector.tensor_scalar(ge[:, NTC - nh:],
                                            cnt[:, NTC - nh:],
                                            float(2 * cfg.K - D), None,
                                            op0=ALU.is_ge)
                else:
                    nc.vector.tensor_scalar(ge[:], cnt[:], float(cfg.K), None,
                                            op0=ALU.is_ge)
                nc.vector.tensor_tensor(dif[:], mid[:], lo[:],
                                        op=ALU.subtract)
                nc.vector.tensor_tensor(dif[:], ge[:], dif[:], op=ALU.mult)
                nc.vector.tensor_tensor(lo[:], lo[:], dif[:], op=ALU.add)
                nc.vector.tensor_tensor(dif[:], hi[:], mid[:],
                                        op=ALU.subtract)
                nc.vector.tensor_tensor(dif[:], ge[:], dif[:], op=ALU.mult)
                nc.vector.tensor_tensor(hi[:], mid[:], dif[:], op=ALU.add)
            # quantize: x8 = round(a * s8) * (|a| >= lo), s8 = 127/m8
            nc.vector.reciprocal(s8[:], m8[:])
            nc.vector.tensor_scalar(s8[:], s8[:], 127.0, None, op0=ALU.mult)
            for j in range(NTC):
                tmp = cpw.tile([128, D], F32, tag="c_tmp")
                nc.vector.tensor_scalar(tmp[:], at[j][:], s8[:, j:j + 1],
                                        MAGIC, op0=ALU.mult, op1=ALU.add)
                nc.vector.tensor_scalar(tmp[:], tmp[:], MAGIC, None,
                                        op0=ALU.subtract)
                msk = cpw.tile([128, D], F32, tag="c_msk")
                nc.vector.tensor_scalar(msk[:], absa[j][:], lo[:, j:j + 1],
                                        None, op0=ALU.is_ge)
                nc.vector.tensor_tensor(x8[j][:], tmp[:], msk[:], op=ALU.mult)
        # --- C2: transpose x8, matmul vs pre-ternarized woT, scale, store ---
        with tc.tile_pool(name="pc2", bufs=1) as cp2, \
             tc.tile_pool(name="pc2w", bufs=3) as cw2, \
             tc.tile_pool(name="pc2_ps", bufs=3, space="PSUM") as cps:
            x8T = []
            for dt in range(NDT):
                pst = cps.tile([128, TPC], BF16, tag="c_pstr")
                for j in range(NTC):
                    nc.tensor.transpose(pst[:, j * 128:(j + 1) * 128],
                                        x8[j][:, dt * 128:(dt + 1) * 128],
                                        idb[:])
                t = cp2.tile([128, TPC], BF16, tag=f"x8T_{dt}",
                             name=f"x8T_{dt}")
                nc.scalar.copy(t[:], pst[:])
                x8T.append(t)
            # y = (x8 @ wot.T) * (s_wo/127) * m8
            ysc = cp2.tile([128, NTC], F32)
            nc.vector.tensor_scalar(ysc[:], m8[:], wsc[:, 1:2], None,
                                    op0=ALU.mult)
            for j in range(NTC):
                ysb = cw2.tile([128, D], F32, tag="c_y")
                for fc in range(NFC):
                    ps = cps.tile([128, 512], F32, tag="c_psy")
                    for dt in range(NDT):
                        nc.tensor.matmul(ps[:],
                                         x8T[dt][:, j * 128:(j + 1) * 128],
                                         wot[dt][:, fc * 512:(fc + 1) * 512],
                                         start=(dt == 0), stop=(dt == NDT - 1))
                    nc.vector.tensor_scalar(ysb[:, fc * 512:(fc + 1) * 512],
                                            ps[:], ysc[:, j:j + 1], None,
                                            op0=ALU.mult)
                nc.sync.dma_start(y_d[j * 128:(j + 1) * 128, :], ysb[:])


# ---------------------------------------------------------------------------
# Host-side driver
# ---------------------------------------------------------------------------
_CACHED = {}


def _get_nc(cfg):
    key = (cfg.B, cfg.T, cfg.D, cfg.H, cfg.HD, cfg.search_iters,
           cfg.attn_f32r, cfg.stop_after)
    if key not in _CACHED:
        _CACHED[key] = build(cfg)
    return _CACHED[key]


def _ternarize(w):
    w = np.asarray(w, np.float32)
    s = np.float32(np.mean(np.abs(w)))
    wi = np.clip(np.round(w / (s + np.float32(EPS))), -1.0, 1.0)
    return s, wi.astype(np.float32)


def _swizzle_qkv(wi, H, HD):
    # w [D_out, D_in] -> wT [D_in, D_out] -> [128, (h t f)] with
    # col ((h*NDT + t)*128 + f) = wT[t*128 + p, h*HD + f]
    D = wi.shape[0]
    wT = np.ascontiguousarray(wi.T)
    NDT = D // 128
    return np.ascontiguousarray(
        wT.reshape(NDT, 128, H, HD).transpose(1, 2, 0, 3).reshape(128, -1)
    ).astype(ml_dtypes.float8_e4m3)


def prep_inputs(cfg, x, wq, wk, wv, wo):
    B, T, D, H, HD = cfg.B, cfg.T, cfg.D, cfg.H, cfg.HD
    TPC = cfg.TPC
    x = np.asarray(x, np.float32).reshape(B, T, D)
    s_q, wq_i = _ternarize(wq)
    s_k, wk_i = _ternarize(wk)
    s_v, wv_i = _ternarize(wv)
    s_o, wo_i = _ternarize(wo)
    wqS = _swizzle_qkv(wq_i, H, HD)
    wkS = _swizzle_qkv(wk_i, H, HD)
    wvS = _swizzle_qkv(wv_i, H, HD)
    woTt = np.ascontiguousarray(wo_i.T).astype(ml_dtypes.bfloat16)
    cos, sin_pm = rope_tables(cfg)
    idf = np.eye(128, dtype=np.float32)
    idb = idf.astype(ml_dtypes.bfloat16)
    wsc = np.zeros((128, 2), np.float32)
    wsc[:, 0] = s_v
    wsc[:, 1] = s_o / 127.0
    in_maps = []
    for c in range(NCORES):
        b, r = divmod(c, T // TPC)
        perm = (np.arange(T) + r * TPC) % T
        in_maps.append({
            "xb": np.ascontiguousarray(x[b][perm]),
            "wqS": wqS, "wkS": wkS, "wvS": wvS, "woTt": woTt,
            "cosk": np.ascontiguousarray(cos[:, perm] * s_k),
            "sink": np.ascontiguousarray(sin_pm[:, perm] * s_k),
            "cosq": np.ascontiguousarray(cos[:, perm[:TPC]] * s_q),
            "sinq": np.ascontiguousarray(sin_pm[:, perm[:TPC]] * s_q),
            "wsc": wsc, "idf": idf, "idb": idb,
        })
    return in_maps


def run(cfg, x, wq, wk, wv, wo, **kw):
    in_maps = prep_inputs(cfg, x, wq, wk, wv, wo)
    nc = _get_nc(cfg)
    res = run_bass_kernel_spmd(nc, in_maps, list(range(NCORES)), **kw)
    T, TPC, D = cfg.T, cfg.TPC, cfg.D
    y = np.empty((cfg.B, T, D), np.float32)
    for c in range(NCORES):
        b, r = divmod(c, T // TPC)
        y[b, r * TPC:(r + 1) * TPC] = res.results[c]["y"]
    return y


def kernel(x, wq, wk, wv, wo):
    return run(Cfg(), x, wq, wk, wv, wo)


if __name__ == "__main__":
    cfg = Cfg()
    rng = np.random.default_rng(0)
    x = rng.standard_normal((cfg.B, cfg.T, cfg.D)).astype(np.float32)
    ws = [(rng.standard_normal((cfg.D, cfg.D)) * 0.02).astype(np.float32)
          for _ in range(4)]
    y = kernel(x, *ws)
    print("out", y.shape, y.dtype, float(np.abs(y).max()))
